# revision 27
# baseline (speedup 1.0000x reference)
"""BiLSTM-CRF (Viterbi decode) Trainium2 Bass kernel.

Sharding: direction-split x batch-split over 8 cores.
  - Pair p in {0,1,2,3}: cores p (forward LSTM) and p+4 (backward LSTM, fed
    time-reversed tokens) both own sentences [8p, 8p+8).
  - Each core: gathers embeddings on-device (indirect DMA), precomputes the
    input contribution Zx = W_ih @ x_t for all t (bf16 matmul), runs the
    sequential LSTM recurrence with stationary bf16 weights (gates land
    transposed: [gate_dim on partitions, batch on free] -> no per-step
    transpose), projects to CRF emission scores, and AllGathers the partial
    emissions within its pair.
  - Viterbi runs WITHOUT traceback: a forward max-plus scan and a backward
    max-plus scan (16 jobs in one set of [16, 144] DVE ops), then
    path[t] = argmax_y(fwd[t,y] + bwd[T-1-t,y] - emit[t,y]) fully in parallel.
    Both cores of a pair redundantly compute all 8 sentences; host reads
    outputs from cores 0-3.
"""

import os
import sys

for _p in ("/opt/trn_rl_repo", "/root/.axon_site/_ro/trn_rl_repo"):
    if os.path.isdir(_p) and _p not in sys.path:
        sys.path.append(_p)

import numpy as np
import ml_dtypes

import concourse.bass as bass
import concourse.tile as tile
from concourse import bacc, mybir
from concourse.bass import AP, IndirectOffsetOnAxis
from concourse.bass_utils import run_bass_kernel_spmd
from concourse.masks import make_identity

F32 = mybir.dt.float32
BF16 = mybir.dt.bfloat16
I32 = mybir.dt.int32
AF = mybir.ActivationFunctionType
ALU = mybir.AluOpType

VOCAB = 100000
EMB = 256
HID = 512
HH = 256  # per-direction hidden
NTAGS = 12
START_IDX = 10
STOP_IDX = 11
NEG = -10000.0

B = 32
T = 256
NCORES = 8
BC = 8          # sentences per pair/core
TOK = T * BC    # 2048 tokens per core
KE = 2          # emb k-tiles
KH = 2          # hidden k-tiles
M = 8           # gate m-tiles (4*HH/128)
J = 40          # viterbi scan partition span: fwd jobs rows 0-7, bwd jobs rows 32-39
BOFF = 32       # partition offset of backward jobs (engine APs need 32-aligned starts)
HSLOT = T + 1   # h history slots (slot 0 = h0)
HFREE = KH * HSLOT * 8  # h_allT free size

# gate order stays (i, f, g, o); with the tanh-trick no contiguity is needed
GATE_PERM = np.arange(1024)

REPLICA_GROUPS = [[0, 4], [1, 5], [2, 6], [3, 7]]


def _ap(t_ap, offset, pattern):
    """New AP over the same tensor with explicit free pattern (keeps partition dim)."""
    return AP(t_ap.tensor, offset, [list(t_ap.ap[0])] + [list(p) for p in pattern])


def build_program(fake_cc=False, num_devices=NCORES, upto=99):
    nc = bacc.Bacc("TRN2", target_bir_lowering=False, debug=False,
                   enable_asserts=False, num_devices=num_devices)

    # ---- I/O ----
    embed_d = nc.dram_tensor("embed", [VOCAB, EMB], F32, kind="ExternalInput")
    idx_d = nc.dram_tensor("idx", [128, 16], I32, kind="ExternalInput")
    wihT_d = nc.dram_tensor("wihT", [128, 2048], BF16, kind="ExternalInput")
    whhT_d = nc.dram_tensor("whhT", [128, 2048], BF16, kind="ExternalInput")
    woutT_d = nc.dram_tensor("woutT", [128, 24], BF16, kind="ExternalInput")
    bT_d = nc.dram_tensor("bT", [128, M], F32, kind="ExternalInput")
    bout_d = nc.dram_tensor("bout_rep", [128, NTAGS], F32, kind="ExternalInput")
    h0T_d = nc.dram_tensor("h0T", [128, 16], BF16, kind="ExternalInput")
    c0T_d = nc.dram_tensor("c0T", [128, 16], F32, kind="ExternalInput")
    trans_d = nc.dram_tensor("trans_dual", [J, 144], F32, kind="ExternalInput")
    init_d = nc.dram_tensor("init_fv", [J, NTAGS], F32, kind="ExternalInput")

    scores_d = nc.dram_tensor("out_scores", [BC], F32, kind="ExternalOutput")
    paths_d = nc.dram_tensor("out_paths", [BC, T], I32, kind="ExternalOutput")

    # collective bounce buffers
    cc_in = nc.dram_tensor("cc_in", [TOK * NTAGS], F32, kind="Internal")
    cc_out = nc.dram_tensor("cc_out", [2 * TOK * NTAGS], F32, kind="Internal")

    from contextlib import ExitStack
    with tile.TileContext(nc) as tc, ExitStack() as ctx:
        _emit(ctx, tc, dict(embed=embed_d, idx=idx_d, wihT=wihT_d, whhT=whhT_d,
                            woutT=woutT_d, bT=bT_d, bout=bout_d, h0T=h0T_d,
                            c0T=c0T_d, trans=trans_d, init=init_d,
                            scores=scores_d, paths=paths_d,
                            cc_in=cc_in, cc_out=cc_out), fake_cc=fake_cc, upto=upto)
    nc.compile()
    return nc


def _emit(ctx, tc, io, fake_cc=False, upto=99):
    nc = tc.nc

    const = ctx.enter_context(tc.tile_pool(name="const", bufs=1))

    ident = const.tile([128, 128], F32, tag="ident")
    make_identity(nc, ident[:])

    # iota over y, [8, T, NTAGS] (values 0..11 are exact in f32)
    iota_f = const.tile([BC, T * NTAGS], F32, tag="iota_f")
    nc.gpsimd.iota(iota_f[:].rearrange("p (t y) -> p t y", y=NTAGS),
                   pattern=[[0, T], [1, NTAGS]], base=0, channel_multiplier=0,
                   allow_small_or_imprecise_dtypes=True)

    # ---- load params ----
    wihT = const.tile([128, 2048], BF16, tag="wihT")
    nc.sync.dma_start(wihT[:], io["wihT"].ap())
    whhT = const.tile([128, 2048], BF16, tag="whhT")
    nc.sync.dma_start(whhT[:], io["whhT"].ap())
    woutT = const.tile([128, 24], BF16, tag="woutT")
    nc.sync.dma_start(woutT[:], io["woutT"].ap())
    bT = const.tile([128, M], F32, tag="bT")
    nc.sync.dma_start(bT[:], io["bT"].ap())
    bout = const.tile([128, NTAGS], F32, tag="bout")
    nc.sync.dma_start(bout[:], io["bout"].ap())
    idx_sb = const.tile([128, 16], I32, tag="idx")
    nc.sync.dma_start(idx_sb[:], io["idx"].ap())
    trans_sb = const.tile([J, 144], F32, tag="trans")
    nc.sync.dma_start(trans_sb[:], io["trans"].ap())
    c0T = const.tile([128, 16], F32, tag="c0T")
    nc.sync.dma_start(c0T[:], io["c0T"].ap())

    # h history: [128, KH * HSLOT * 8] bf16; slot 0 = h0
    hall_pool = ctx.enter_context(tc.tile_pool(name="hall", bufs=1))
    h_allT = hall_pool.tile([128, HFREE], BF16, tag="h_allT")
    nc.sync.dma_start(
        _ap(h_allT[:], 0, [[HSLOT * 8, KH], [1, 8]]),
        AP(io["h0T"], 0, [[16, 128], [8, KH], [1, 8]]))

    # ---------------- phases 1-4 (pools released before Viterbi) ----------------
    zx_ctx = tc.tile_pool(name="zx", bufs=1)
    zx_pool = zx_ctx.__enter__()
    zxB = zx_pool.tile([128, M * TOK], F32, tag="zxB")
    with tc.tile_pool(name="xpool", bufs=1) as xpool, \
         tc.tile_pool(name="xt", bufs=1) as xtpool:
        xrows = xpool.tile([128, 16 * EMB], F32, tag="xrows")
        for j in range(16):
            nc.gpsimd.indirect_dma_start(
                out=xrows[:, j * EMB:(j + 1) * EMB],
                out_offset=None,
                in_=io["embed"].ap(),
                in_offset=IndirectOffsetOnAxis(ap=idx_sb[:, j:j + 1], axis=0))

        # phase 2: transpose to XT [emb(part, 2 ktiles), token] bf16
        xt = xtpool.tile([128, KE * TOK], BF16, tag="xt")
        with tc.tile_pool(name="tp_ps", bufs=4, space="PSUM") as tp_ps:
            for j in range(16):
                for h in range(KE):
                    ps = tp_ps.tile([128, 128], F32, tag="tp")
                    nc.tensor.transpose(
                        out=ps[:], in_=xrows[:, j * EMB + h * 128: j * EMB + (h + 1) * 128],
                        identity=ident[:])
                    nc.vector.tensor_copy(
                        xt[:, h * TOK + j * 128: h * TOK + (j + 1) * 128], ps[:])

        if upto < 3:
            nc.gpsimd.dma_start(AP(io["paths"], 0, [[1, 64]]), xt[0:1, 0:64])
            return
        # phase 3: ZxB = W_ih @ X^T + b   [gate(part, 8 mtiles), token] f32
        with tc.tile_pool(name="zx_ps", bufs=4, space="PSUM") as zx_ps:
            for m in range(M):
                for n in range(4):
                    ps = zx_ps.tile([128, 512], F32, tag="zx")
                    for k in range(KE):
                        nc.tensor.matmul(
                            out=ps[:],
                            lhsT=wihT[:, (k * M + m) * 128:(k * M + m + 1) * 128],
                            rhs=xt[:, k * TOK + n * 512: k * TOK + (n + 1) * 512],
                            start=(k == 0), stop=(k == KE - 1))
                    dst = zxB[:, m * TOK + n * 512: m * TOK + (n + 1) * 512]
                    if (m * 4 + n) % 2 == 0:
                        nc.scalar.activation(out=dst, in_=ps[:], func=AF.Identity,
                                             bias=bT[:, m:m + 1], scale=1.0)
                    else:
                        nc.vector.tensor_scalar_add(dst, ps[:], bT[:, m:m + 1])

    if upto < 4:
        nc.gpsimd.dma_start(AP(io["paths"], 0, [[1, 64]]), zxB[0:1, 0:64])
        return
    # ---------------- phase 4: LSTM recurrence ----------------
    rec_ctxs = [tc.tile_pool(name="rec_ps", bufs=2, space="PSUM"),
                tc.tile_pool(name="gates", bufs=3),
                tc.tile_pool(name="small", bufs=4),
                tc.tile_pool(name="cpool", bufs=3)]
    rec_ps, gates, small, cpool = [c.__enter__() for c in rec_ctxs]

    c_prev = cpool.tile([128, 16], F32, tag="c")
    nc.vector.tensor_copy(c_prev[:], c0T[:])

    zx_t = zxB[:].rearrange("p (m t b) -> p m t b", m=M, b=BC)
    for t in range(T):
        ps = rec_ps.tile([128, M * BC], F32, tag="rec")
        for m in range(M):
            for k in range(KH):
                nc.tensor.matmul(
                    out=ps[:, m * BC:(m + 1) * BC],
                    lhsT=whhT[:, (k * M + m) * 128:(k * M + m + 1) * 128],
                    rhs=h_allT[:, k * HSLOT * 8 + t * 8: k * HSLOT * 8 + (t + 1) * 8],
                    start=(k == 0), stop=(k == KH - 1))
        # gates via ONE tanh per half: i,f,o rows pre-scaled by 1/2 host-side, so
        # tanh(z/2) -> sigmoid(z) = (th+1)/2; g is a plain tanh. The cell state is
        # kept as v = 2c and hidden as h' = 2h (weights pre-scaled), which turns
        # every product into a fused (x+1)*y / (0.5x+y) scalar_tensor_tensor op.
        # The o-gate half (m-tiles 6,7) is processed separately so its add+tanh
        # overlaps the i/f/g -> v -> tanh(c) chain.
        gp = gates.tile([128, 48], F32, tag="gp")
        nc.vector.tensor_add(
            gp[:, 0:48].rearrange("p (m b) -> p m b", b=BC),
            ps[:, 0:48].rearrange("p (m b) -> p m b", b=BC),
            zx_t[:, 0:6, t, :])
        th = gates.tile([128, 48], F32, tag="th")
        nc.scalar.activation(out=th[:, 0:48], in_=gp[:, 0:48], func=AF.Tanh)
        u1 = small.tile([128, 16], F32, tag="u1")
        nc.vector.scalar_tensor_tensor(out=u1[:], in0=th[:, 16:32], scalar=1.0,
                                       in1=c_prev[:], op0=ALU.add, op1=ALU.mult)
        u2 = small.tile([128, 16], F32, tag="u2")
        nc.vector.scalar_tensor_tensor(out=u2[:], in0=th[:, 0:16], scalar=1.0,
                                       in1=th[:, 32:48], op0=ALU.add, op1=ALU.mult)
        c_new = cpool.tile([128, 16], F32, tag="c")
        nc.vector.scalar_tensor_tensor(out=c_new[:], in0=u1[:], scalar=0.5,
                                       in1=u2[:], op0=ALU.mult, op1=ALU.add)
        tc_t = small.tile([128, 16], F32, tag="tc")
        nc.scalar.activation(out=tc_t[:], in_=c_new[:], func=AF.Tanh, scale=0.5)
        gp2 = small.tile([128, 16], F32, tag="gp2")
        nc.vector.tensor_add(
            gp2[:].rearrange("p (m b) -> p m b", b=BC),
            ps[:, 48:64].rearrange("p (m b) -> p m b", b=BC),
            zx_t[:, 6:8, t, :])
        th2 = small.tile([128, 16], F32, tag="th2")
        nc.scalar.activation(out=th2[:], in_=gp2[:], func=AF.Tanh)
        hout = _ap(h_allT[:], (t + 1) * 8, [[HSLOT * 8, KH], [1, 8]])
        nc.vector.scalar_tensor_tensor(
            out=hout,
            in0=th2[:].rearrange("p (k b) -> p k b", b=8), scalar=1.0,
            in1=tc_t[:].rearrange("p (k b) -> p k b", b=8),
            op0=ALU.add, op1=ALU.mult)
        c_prev = c_new

    for c in reversed(rec_ctxs):
        c.__exit__(None, None, None)
    zx_ctx.__exit__(None, None, None)

    if upto < 5:
        nc.gpsimd.dma_start(AP(io["paths"], 0, [[1, 64]]), h_allT[0:1, 0:128].bitcast(I32))
        return
    # ---------------- phase 5: emission partials + AllGather ----------------
    vit = ctx.enter_context(tc.tile_pool(name="vit", bufs=1))
    feats_sb = vit.tile([128, 16 * NTAGS], F32, tag="feats")
    with tc.tile_pool(name="f_ps", bufs=4, space="PSUM") as f_ps:
        for mc in range(16):
            ps = f_ps.tile([128, NTAGS], F32, tag="f")
            for k in range(KH):
                nc.tensor.matmul(
                    out=ps[:],
                    lhsT=h_allT[:, k * HSLOT * 8 + 8 + mc * 128: k * HSLOT * 8 + 8 + (mc + 1) * 128],
                    rhs=woutT[:, k * NTAGS:(k + 1) * NTAGS],
                    start=(k == 0), stop=(k == KH - 1))
            nc.vector.tensor_add(feats_sb[:, mc * NTAGS:(mc + 1) * NTAGS],
                                 ps[:], bout[:])

    # SBUF [128 p, (chunk 16, y 12)] -> DRAM flat [(chunk*128+p) * 12 + y]
    nc.sync.dma_start(
        AP(io["cc_in"], 0, [[NTAGS, 128], [128 * NTAGS, 16], [1, NTAGS]]),
        feats_sb[:].rearrange("p (c y) -> p c y", y=NTAGS))
    if fake_cc:
        # single-core timing-model variant: stand in for the pair AllGather
        half = TOK * NTAGS
        nc.sync.dma_start(AP(io["cc_out"], 0, [[1, half]]), io["cc_in"].ap())
        nc.sync.dma_start(AP(io["cc_out"], half, [[1, half]]), io["cc_in"].ap())
    else:
        nc.gpsimd.collective_compute(
            "AllGather", ALU.bypass, replica_groups=REPLICA_GROUPS,
            ins=[io["cc_in"].ap()], outs=[io["cc_out"].ap()])

    # ---------------- phase 6: build emit_sb [J, T*12] ----------------
    # G = rank0 partial (global t order), R = rank1 partial (reversed t order)
    s1g = vit.tile([J, T * NTAGS], F32, tag="s1g")
    s1r = vit.tile([J, T * NTAGS], F32, tag="s1r")
    goff, roff = 0, TOK * NTAGS
    dram_pat = [[NTAGS, BC], [BC * NTAGS, T], [1, NTAGS]]  # (b, t, y)
    nc.sync.dma_start(s1g[0:8, :], AP(io["cc_out"], goff, dram_pat))
    nc.sync.dma_start(s1g[BOFF:BOFF + 8, :], AP(io["cc_out"], goff, dram_pat))
    nc.sync.dma_start(s1r[0:8, :], AP(io["cc_out"], roff, dram_pat))
    nc.sync.dma_start(s1r[BOFF:BOFF + 8, :], AP(io["cc_out"], roff, dram_pat))

    emit_sb = vit.tile([J, T * NTAGS], F32, tag="emit")
    nc.vector.memset(emit_sb[:], 0.0)

    def rev_t(ap2d, nrows):
        # view [rows, t, y] with t reversed
        return AP(ap2d.tensor, ap2d.offset + (T - 1) * NTAGS,
                  [list(ap2d.ap[0]), [-NTAGS, T], [1, NTAGS]])

    v3 = lambda a: a.rearrange("p (t y) -> p t y", y=NTAGS)
    nc.vector.tensor_add(v3(emit_sb[0:8, :]), v3(s1g[0:8, :]), rev_t(s1r[0:8, :], 8))
    nc.vector.tensor_add(v3(emit_sb[BOFF:BOFF + 8, :]), rev_t(s1g[BOFF:BOFF + 8, :], 8), v3(s1r[BOFF:BOFF + 8, :]))

    # ---------------- phase 7: the two max-plus scans ----------------
    fv_all = vit.tile([J, (T + 1) * NTAGS], F32, tag="fv_all")
    nc.vector.memset(fv_all[:], NEG)
    nc.sync.dma_start(fv_all[:, 0:NTAGS], io["init"].ap())

    CH = 32  # A-table chunk length (t steps)
    with tc.tile_pool(name="scanA", bufs=2) as apool, \
         tc.tile_pool(name="scan", bufs=3) as scan_pool:
        for c0_ in range(0, T, CH):
            # A[j, t, next, prev] = trans_dual[j, next, prev] + emit[j, t, next]
            A = apool.tile([J, CH * 144], F32, tag="A")
            nc.vector.tensor_tensor(
                out=_ap(A[:], 0, [[144, CH], [NTAGS, NTAGS], [1, NTAGS]]),
                in0=_ap(trans_sb[:], 0, [[0, CH], [NTAGS, NTAGS], [1, NTAGS]]),
                in1=_ap(emit_sb[:], c0_ * NTAGS, [[NTAGS, CH], [1, NTAGS], [0, NTAGS]]),
                op=ALU.add)
            for tt in range(CH):
                t = c0_ + tt
                m_t = scan_pool.tile([J, 144], F32, tag="m")
                fv_b = _ap(fv_all[:], t * NTAGS, [[0, NTAGS], [1, NTAGS]])
                nc.vector.tensor_add(
                    m_t[:].rearrange("p (a b) -> p a b", b=NTAGS), fv_b,
                    _ap(A[:], tt * 144, [[NTAGS, NTAGS], [1, NTAGS]]))
                nc.vector.reduce_max(fv_all[:, (t + 1) * NTAGS:(t + 2) * NTAGS],
                                     m_t[:].rearrange("p (a b) -> p a b", b=NTAGS),
                                     axis=mybir.AxisListType.X)

    if upto < 8:
        nc.gpsimd.dma_start(AP(io["paths"], 0, [[1, 64]]), fv_all[0:1, 0:64])
        return
    # ---------------- phase 8: finalize ----------------
    u2 = vit.tile([BC, T * NTAGS], F32, tag="u2")
    nc.sync.dma_start(u2[:], fv_all[BOFF:BOFF + 8, NTAGS:(T + 1) * NTAGS])

    tA = vit.tile([BC, T * NTAGS], F32, tag="tA")
    nc.vector.tensor_sub(tA[:], fv_all[0:8, NTAGS:(T + 1) * NTAGS], emit_sb[0:8, :])
    tot = vit.tile([BC, T * NTAGS], F32, tag="tot")
    nc.vector.tensor_add(v3(tot[:]), v3(tA[:]), rev_t(u2[:], 8))

    maxv = vit.tile([BC, T], F32, tag="maxv")
    nc.vector.reduce_max(maxv[:], v3(tot[:]), axis=mybir.AxisListType.X)

    eq = vit.tile([BC, T * NTAGS], F32, tag="s1g")
    nc.vector.tensor_tensor(
        out=v3(eq[:]), in0=v3(tot[:]),
        in1=_ap(maxv[:], 0, [[1, T], [0, NTAGS]]),
        op=ALU.is_ge)
    masked = vit.tile([BC, T * NTAGS], F32, tag="s1r")
    nc.vector.scalar_tensor_tensor(
        out=masked[:], in0=eq[:], scalar=1.0e6, in1=iota_f[:],
        op0=ALU.mult, op1=ALU.subtract)
    pm = vit.tile([BC, T], F32, tag="pm")
    nc.vector.reduce_max(pm[:], v3(masked[:]), axis=mybir.AxisListType.X)
    pathf = vit.tile([BC, T], F32, tag="pathf")
    nc.vector.tensor_scalar(out=pathf[:], in0=pm[:], scalar1=-1.0, scalar2=1.0e6,
                            op0=ALU.mult, op1=ALU.add)
    paths_sb = vit.tile([BC, T], I32, tag="paths")
    nc.vector.tensor_copy(paths_sb[:], pathf[:])
    scores_sb = vit.tile([BC, 1], F32, tag="scores")
    nc.vector.tensor_copy(scores_sb[:], maxv[:, 0:1])

    nc.sync.dma_start(io["paths"].ap(), paths_sb[:])
    nc.sync.dma_start(AP(io["scores"], 0, [[1, BC], [1, 1]]), scores_sb[:])


# ---------------------------------------------------------------------------
# host side
# ---------------------------------------------------------------------------

def _wT_blocks(w):
    """[4HH, K] weight -> [128, (ktiles*mtiles)*128] lhsT block layout."""
    kk = w.shape[1] // 128
    mm = w.shape[0] // 128
    wt = w.T.reshape(kk, 128, mm, 128)          # [k, r, m, c]
    return np.ascontiguousarray(wt.transpose(1, 0, 2, 3).reshape(128, kk * mm * 128))


def _make_inputs(inputs):
    sent = np.asarray(inputs["sentence"]).astype(np.int32)
    embed = np.asarray(inputs["embed"]).astype(np.float32)
    h0 = np.asarray(inputs["h0"]).astype(np.float32)
    c0 = np.asarray(inputs["c0"]).astype(np.float32)
    trans = np.asarray(inputs["transitions"]).astype(np.float32)
    b_out = np.asarray(inputs["b_out"]).astype(np.float32)
    W_out = np.asarray(inputs["W_out"]).astype(np.float32)

    Wih = [np.asarray(inputs["W_ih_f"]), np.asarray(inputs["W_ih_b"])]
    Whh = [np.asarray(inputs["W_hh_f"]), np.asarray(inputs["W_hh_b"])]
    bb = [np.asarray(inputs["b_f"]), np.asarray(inputs["b_b"])]

    init_fv = np.full((J, NTAGS), NEG, np.float32)
    init_fv[0:8, START_IDX] = 0.0
    init_fv[32:40, STOP_IDX] = 0.0
    trans_dual = np.tile(trans.reshape(1, 144), (J, 1)).astype(np.float32)
    trans_dual[32:40] = np.tile(trans.T.reshape(1, 144), (8, 1))

    in_maps = []
    for c in range(NCORES):
        d = 1 if c >= 4 else 0
        p = c % 4
        sl = slice(8 * p, 8 * p + 8)
        s = sent[sl]
        if d == 1:
            s = s[:, ::-1]
        flat = np.ascontiguousarray(s.T).reshape(TOK)          # t-major, b-minor
        idx = np.ascontiguousarray(flat.reshape(16, 128).T).astype(np.int32)

        # tanh-trick scaling: i,f,o rows x1/2 (sigmoid via tanh); and the
        # device carries h' = 2h, v = 2c -> W_hh cols x1/2, W_out x1/2, h0 x2, c0 x2.
        rs = np.full((1024, 1), 0.5, np.float32)
        rs[512:768] = 1.0  # g rows keep full scale (plain tanh)
        wih = Wih[d][GATE_PERM] * rs
        whh = Whh[d][GATE_PERM] * rs * 0.5
        bvec = bb[d][GATE_PERM] * rs[:, 0]
        h0s = h0[d][sl] * 2.0   # [8, HH]
        c0s = c0[d][sl] * 2.0
        h0T = np.ascontiguousarray(h0s.T.reshape(KH, 128, 8).transpose(1, 0, 2)
                                   .reshape(128, 16))
        c0T = np.ascontiguousarray(c0s.T.reshape(KH, 128, 8).transpose(1, 0, 2)
                                   .reshape(128, 16))
        wout = W_out[:, d * HH:(d + 1) * HH] * 0.5  # [12, 256]; x1/2 since h'=2h
        woutT = np.ascontiguousarray(wout.T.reshape(KH, 128, NTAGS)
                                     .transpose(1, 0, 2).reshape(128, KH * NTAGS))
        bout_rep = (np.tile(b_out, (128, 1)) if d == 0
                    else np.zeros((128, NTAGS))).astype(np.float32)

        in_maps.append({
            "embed": embed,
            "idx": idx,
            "wihT": _wT_blocks(wih).astype(ml_dtypes.bfloat16),
            "whhT": _wT_blocks(whh).astype(ml_dtypes.bfloat16),
            "woutT": woutT.astype(ml_dtypes.bfloat16),
            "bT": np.ascontiguousarray(bvec.reshape(M, 128).T).astype(np.float32),
            "bout_rep": bout_rep,
            "h0T": h0T.astype(ml_dtypes.bfloat16),
            "c0T": c0T.astype(np.float32),
            "trans_dual": trans_dual,
            "init_fv": init_fv,
        })
    return in_maps


_NC_CACHE = None


def _get_nc():
    global _NC_CACHE
    if _NC_CACHE is None:
        _NC_CACHE = build_program()
    return _NC_CACHE


def kernel(**inputs):
    nc = _get_nc()
    in_maps = _make_inputs(inputs)
    res = run_bass_kernel_spmd(nc, in_maps, core_ids=list(range(NCORES)))
    scores = np.zeros(B, np.float32)
    paths = np.zeros((B, T), np.int32)
    for p in range(4):
        r = res.results[p]
        scores[8 * p:8 * p + 8] = r["out_scores"]
        paths[8 * p:8 * p + 8] = r["out_paths"]
    return scores, paths


if __name__ == "__main__":
    nc = _get_nc()
    print("program built + compiled OK")


# revision 28
# speedup vs baseline: 1.0354x; 1.0354x over previous
"""BiLSTM-CRF (Viterbi decode) Trainium2 Bass kernel.

Sharding: direction-split x batch-split over 8 cores.
  - Pair p in {0,1,2,3}: cores p (forward LSTM) and p+4 (backward LSTM, fed
    time-reversed tokens) both own sentences [8p, 8p+8).
  - Each core: gathers embeddings on-device (indirect DMA), precomputes the
    input contribution Zx = W_ih @ x_t for all t (bf16 matmul), runs the
    sequential LSTM recurrence with stationary bf16 weights (gates land
    transposed: [gate_dim on partitions, batch on free] -> no per-step
    transpose), projects to CRF emission scores, and AllGathers the partial
    emissions within its pair.
  - Viterbi runs WITHOUT traceback: a forward max-plus scan and a backward
    max-plus scan (16 jobs in one set of [16, 144] DVE ops), then
    path[t] = argmax_y(fwd[t,y] + bwd[T-1-t,y] - emit[t,y]) fully in parallel.
    Both cores of a pair redundantly compute all 8 sentences; host reads
    outputs from cores 0-3.
"""

import os
import sys

for _p in ("/opt/trn_rl_repo", "/root/.axon_site/_ro/trn_rl_repo"):
    if os.path.isdir(_p) and _p not in sys.path:
        sys.path.append(_p)

import numpy as np
import ml_dtypes

import concourse.bass as bass
import concourse.tile as tile
from concourse import bacc, mybir
from concourse.bass import AP, IndirectOffsetOnAxis
from concourse.bass_utils import run_bass_kernel_spmd
from concourse.masks import make_identity

F32 = mybir.dt.float32
BF16 = mybir.dt.bfloat16
I32 = mybir.dt.int32
AF = mybir.ActivationFunctionType
ALU = mybir.AluOpType

VOCAB = 100000
EMB = 256
HID = 512
HH = 256  # per-direction hidden
NTAGS = 12
START_IDX = 10
STOP_IDX = 11
NEG = -10000.0

B = 32
T = 256
NCORES = 8
BC = 8          # sentences per pair/core
TOK = T * BC    # 2048 tokens per core
KE = 2          # emb k-tiles
KH = 2          # hidden k-tiles
M = 8           # gate m-tiles (4*HH/128)
J = 40          # viterbi scan partition span: fwd jobs rows 0-7, bwd jobs rows 32-39
BOFF = 32       # partition offset of backward jobs (engine APs need 32-aligned starts)
HSLOT = T + 1   # h history slots (slot 0 = h0)
HFREE = KH * HSLOT * 8  # h_allT free size

# gate order stays (i, f, g, o); with the tanh-trick no contiguity is needed
GATE_PERM = np.arange(1024)

REPLICA_GROUPS = [[0, 4], [1, 5], [2, 6], [3, 7]]


def _ap(t_ap, offset, pattern):
    """New AP over the same tensor with explicit free pattern (keeps partition dim)."""
    return AP(t_ap.tensor, offset, [list(t_ap.ap[0])] + [list(p) for p in pattern])


def build_program(fake_cc=False, num_devices=NCORES, upto=99):
    nc = bacc.Bacc("TRN2", target_bir_lowering=False, debug=False,
                   enable_asserts=False, num_devices=num_devices)

    # ---- I/O ----
    embed_d = nc.dram_tensor("embed", [VOCAB, EMB], F32, kind="ExternalInput")
    idx_d = nc.dram_tensor("idx", [128, 16], I32, kind="ExternalInput")
    wihT_d = nc.dram_tensor("wihT", [128, 2048], BF16, kind="ExternalInput")
    whhT_d = nc.dram_tensor("whhT", [128, 2048], BF16, kind="ExternalInput")
    woutT_d = nc.dram_tensor("woutT", [128, 24], BF16, kind="ExternalInput")
    bT_d = nc.dram_tensor("bT", [128, M], F32, kind="ExternalInput")
    bout_d = nc.dram_tensor("bout_rep", [128, NTAGS], F32, kind="ExternalInput")
    h0T_d = nc.dram_tensor("h0T", [128, 16], BF16, kind="ExternalInput")
    c0T_d = nc.dram_tensor("c0T", [128, 16], F32, kind="ExternalInput")
    trans_d = nc.dram_tensor("trans_dual", [J, 144], F32, kind="ExternalInput")
    init_d = nc.dram_tensor("init_fv", [J, NTAGS], F32, kind="ExternalInput")

    scores_d = nc.dram_tensor("out_scores", [BC], F32, kind="ExternalOutput")
    paths_d = nc.dram_tensor("out_paths", [BC, T], I32, kind="ExternalOutput")

    # collective bounce buffers
    cc_in = nc.dram_tensor("cc_in", [TOK * NTAGS], F32, kind="Internal")
    cc_out = nc.dram_tensor("cc_out", [2 * TOK * NTAGS], F32, kind="Internal")

    from contextlib import ExitStack
    with tile.TileContext(nc) as tc, ExitStack() as ctx:
        _emit(ctx, tc, dict(embed=embed_d, idx=idx_d, wihT=wihT_d, whhT=whhT_d,
                            woutT=woutT_d, bT=bT_d, bout=bout_d, h0T=h0T_d,
                            c0T=c0T_d, trans=trans_d, init=init_d,
                            scores=scores_d, paths=paths_d,
                            cc_in=cc_in, cc_out=cc_out), fake_cc=fake_cc, upto=upto)
    nc.compile()
    return nc


def _emit(ctx, tc, io, fake_cc=False, upto=99):
    nc = tc.nc

    const = ctx.enter_context(tc.tile_pool(name="const", bufs=1))

    ident = const.tile([128, 128], F32, tag="ident")
    make_identity(nc, ident[:])

    # iota over y, [8, T, NTAGS] (values 0..11 are exact in f32)
    iota_f = const.tile([BC, T * NTAGS], F32, tag="iota_f")
    nc.gpsimd.iota(iota_f[:].rearrange("p (t y) -> p t y", y=NTAGS),
                   pattern=[[0, T], [1, NTAGS]], base=0, channel_multiplier=0,
                   allow_small_or_imprecise_dtypes=True)

    # ---- load params ----
    wihT = const.tile([128, 2048], BF16, tag="wihT")
    nc.sync.dma_start(wihT[:], io["wihT"].ap())
    whhT = const.tile([128, 2048], BF16, tag="whhT")
    nc.sync.dma_start(whhT[:], io["whhT"].ap())
    woutT = const.tile([128, 24], BF16, tag="woutT")
    nc.sync.dma_start(woutT[:], io["woutT"].ap())
    bT = const.tile([128, M], F32, tag="bT")
    nc.sync.dma_start(bT[:], io["bT"].ap())
    bout = const.tile([128, NTAGS], F32, tag="bout")
    nc.sync.dma_start(bout[:], io["bout"].ap())
    idx_sb = const.tile([128, 16], I32, tag="idx")
    nc.sync.dma_start(idx_sb[:], io["idx"].ap())
    trans_sb = const.tile([J, 144], F32, tag="trans")
    nc.sync.dma_start(trans_sb[:], io["trans"].ap())
    c0T = const.tile([128, 16], F32, tag="c0T")
    nc.sync.dma_start(c0T[:], io["c0T"].ap())

    # h history: [128, KH * HSLOT * 8] bf16; slot 0 = h0
    hall_pool = ctx.enter_context(tc.tile_pool(name="hall", bufs=1))
    h_allT = hall_pool.tile([128, HFREE], BF16, tag="h_allT")
    nc.sync.dma_start(
        _ap(h_allT[:], 0, [[HSLOT * 8, KH], [1, 8]]),
        AP(io["h0T"], 0, [[16, 128], [8, KH], [1, 8]]))

    # ---------------- phases 1-4 (pools released before Viterbi) ----------------
    zx_ctx = tc.tile_pool(name="zx", bufs=1)
    zx_pool = zx_ctx.__enter__()
    zxB = zx_pool.tile([128, M * TOK], F32, tag="zxB")
    with tc.tile_pool(name="xpool", bufs=1) as xpool, \
         tc.tile_pool(name="xt", bufs=1) as xtpool:
        xrows = xpool.tile([128, 16 * EMB], F32, tag="xrows")
        for j in range(16):
            nc.gpsimd.indirect_dma_start(
                out=xrows[:, j * EMB:(j + 1) * EMB],
                out_offset=None,
                in_=io["embed"].ap(),
                in_offset=IndirectOffsetOnAxis(ap=idx_sb[:, j:j + 1], axis=0))

        # phase 2: transpose to XT [emb(part, 2 ktiles), token] bf16
        xt = xtpool.tile([128, KE * TOK], BF16, tag="xt")
        with tc.tile_pool(name="tp_ps", bufs=4, space="PSUM") as tp_ps:
            for j in range(16):
                for h in range(KE):
                    ps = tp_ps.tile([128, 128], F32, tag="tp")
                    nc.tensor.transpose(
                        out=ps[:], in_=xrows[:, j * EMB + h * 128: j * EMB + (h + 1) * 128],
                        identity=ident[:])
                    nc.vector.tensor_copy(
                        xt[:, h * TOK + j * 128: h * TOK + (j + 1) * 128], ps[:])

        if upto < 3:
            nc.gpsimd.dma_start(AP(io["paths"], 0, [[1, 64]]), xt[0:1, 0:64])
            return
        # phase 3: ZxB = W_ih @ X^T + b   [gate(part, 8 mtiles), token] f32
        with tc.tile_pool(name="zx_ps", bufs=4, space="PSUM") as zx_ps:
            for m in range(M):
                for n in range(4):
                    ps = zx_ps.tile([128, 512], F32, tag="zx")
                    for k in range(KE):
                        nc.tensor.matmul(
                            out=ps[:],
                            lhsT=wihT[:, (k * M + m) * 128:(k * M + m + 1) * 128],
                            rhs=xt[:, k * TOK + n * 512: k * TOK + (n + 1) * 512],
                            start=(k == 0), stop=(k == KE - 1))
                    dst = zxB[:, m * TOK + n * 512: m * TOK + (n + 1) * 512]
                    if (m * 4 + n) % 2 == 0:
                        nc.scalar.activation(out=dst, in_=ps[:], func=AF.Identity,
                                             bias=bT[:, m:m + 1], scale=1.0)
                    else:
                        nc.vector.tensor_scalar_add(dst, ps[:], bT[:, m:m + 1])

    if upto < 4:
        nc.gpsimd.dma_start(AP(io["paths"], 0, [[1, 64]]), zxB[0:1, 0:64])
        return
    # ---------------- phase 4: LSTM recurrence ----------------
    rec_ctxs = [tc.tile_pool(name="rec_ps", bufs=4, space="PSUM"),
                tc.tile_pool(name="gates", bufs=4),
                tc.tile_pool(name="small", bufs=8),
                tc.tile_pool(name="cpool", bufs=4)]
    rec_ps, gates, small, cpool = [c.__enter__() for c in rec_ctxs]

    c_prev = cpool.tile([128, 16], F32, tag="c")
    nc.vector.tensor_copy(c_prev[:], c0T[:])

    zx_t = zxB[:].rearrange("p (m t b) -> p m t b", m=M, b=BC)
    for t in range(T):
        ps = rec_ps.tile([128, M * BC], F32, tag="rec")
        for m in range(M):
            for k in range(KH):
                nc.tensor.matmul(
                    out=ps[:, m * BC:(m + 1) * BC],
                    lhsT=whhT[:, (k * M + m) * 128:(k * M + m + 1) * 128],
                    rhs=h_allT[:, k * HSLOT * 8 + t * 8: k * HSLOT * 8 + (t + 1) * 8],
                    start=(k == 0), stop=(k == KH - 1))
        # gates via ONE tanh per half: i,f,o rows pre-scaled by 1/2 host-side, so
        # tanh(z/2) -> sigmoid(z) = (th+1)/2; g is a plain tanh. The cell state is
        # kept as v = 2c and hidden as h' = 2h (weights pre-scaled), which turns
        # every product into a fused (x+1)*y / (0.5x+y) scalar_tensor_tensor op.
        # The o-gate half (m-tiles 6,7) is processed separately so its add+tanh
        # overlaps the i/f/g -> v -> tanh(c) chain.
        gp = gates.tile([128, 48], F32, tag="gp")
        nc.vector.tensor_add(
            gp[:, 0:48].rearrange("p (m b) -> p m b", b=BC),
            ps[:, 0:48].rearrange("p (m b) -> p m b", b=BC),
            zx_t[:, 0:6, t, :])
        th = gates.tile([128, 48], F32, tag="th")
        nc.scalar.activation(out=th[:, 0:48], in_=gp[:, 0:48], func=AF.Tanh)
        u1 = small.tile([128, 16], F32, tag="u1")
        nc.vector.scalar_tensor_tensor(out=u1[:], in0=th[:, 16:32], scalar=1.0,
                                       in1=c_prev[:], op0=ALU.add, op1=ALU.mult)
        u2 = small.tile([128, 16], F32, tag="u2")
        nc.vector.scalar_tensor_tensor(out=u2[:], in0=th[:, 0:16], scalar=1.0,
                                       in1=th[:, 32:48], op0=ALU.add, op1=ALU.mult)
        c_new = cpool.tile([128, 16], F32, tag="c")
        nc.vector.scalar_tensor_tensor(out=c_new[:], in0=u1[:], scalar=0.5,
                                       in1=u2[:], op0=ALU.mult, op1=ALU.add)
        tc_t = small.tile([128, 16], F32, tag="tc")
        nc.scalar.activation(out=tc_t[:], in_=c_new[:], func=AF.Tanh, scale=0.5)
        gp2 = small.tile([128, 16], F32, tag="gp2")
        nc.vector.tensor_add(
            gp2[:].rearrange("p (m b) -> p m b", b=BC),
            ps[:, 48:64].rearrange("p (m b) -> p m b", b=BC),
            zx_t[:, 6:8, t, :])
        th2 = small.tile([128, 16], F32, tag="th2")
        nc.scalar.activation(out=th2[:], in_=gp2[:], func=AF.Tanh)
        hout = _ap(h_allT[:], (t + 1) * 8, [[HSLOT * 8, KH], [1, 8]])
        nc.vector.scalar_tensor_tensor(
            out=hout,
            in0=th2[:].rearrange("p (k b) -> p k b", b=8), scalar=1.0,
            in1=tc_t[:].rearrange("p (k b) -> p k b", b=8),
            op0=ALU.add, op1=ALU.mult)
        c_prev = c_new

    for c in reversed(rec_ctxs):
        c.__exit__(None, None, None)
    zx_ctx.__exit__(None, None, None)

    if upto < 5:
        nc.gpsimd.dma_start(AP(io["paths"], 0, [[1, 64]]), h_allT[0:1, 0:128].bitcast(I32))
        return
    # ---------------- phase 5: emission partials + AllGather ----------------
    vit = ctx.enter_context(tc.tile_pool(name="vit", bufs=1))
    feats_sb = vit.tile([128, 16 * NTAGS], F32, tag="feats")
    with tc.tile_pool(name="f_ps", bufs=4, space="PSUM") as f_ps:
        for mc in range(16):
            ps = f_ps.tile([128, NTAGS], F32, tag="f")
            for k in range(KH):
                nc.tensor.matmul(
                    out=ps[:],
                    lhsT=h_allT[:, k * HSLOT * 8 + 8 + mc * 128: k * HSLOT * 8 + 8 + (mc + 1) * 128],
                    rhs=woutT[:, k * NTAGS:(k + 1) * NTAGS],
                    start=(k == 0), stop=(k == KH - 1))
            nc.vector.tensor_add(feats_sb[:, mc * NTAGS:(mc + 1) * NTAGS],
                                 ps[:], bout[:])

    # SBUF [128 p, (chunk 16, y 12)] -> DRAM flat [(chunk*128+p) * 12 + y]
    nc.sync.dma_start(
        AP(io["cc_in"], 0, [[NTAGS, 128], [128 * NTAGS, 16], [1, NTAGS]]),
        feats_sb[:].rearrange("p (c y) -> p c y", y=NTAGS))
    if fake_cc:
        # single-core timing-model variant: stand in for the pair AllGather
        half = TOK * NTAGS
        nc.sync.dma_start(AP(io["cc_out"], 0, [[1, half]]), io["cc_in"].ap())
        nc.sync.dma_start(AP(io["cc_out"], half, [[1, half]]), io["cc_in"].ap())
    else:
        nc.gpsimd.collective_compute(
            "AllGather", ALU.bypass, replica_groups=REPLICA_GROUPS,
            ins=[io["cc_in"].ap()], outs=[io["cc_out"].ap()])

    # ---------------- phase 6: build emit_sb [J, T*12] ----------------
    # G = rank0 partial (global t order), R = rank1 partial (reversed t order)
    s1g = vit.tile([J, T * NTAGS], F32, tag="s1g")
    s1r = vit.tile([J, T * NTAGS], F32, tag="s1r")
    goff, roff = 0, TOK * NTAGS
    dram_pat = [[NTAGS, BC], [BC * NTAGS, T], [1, NTAGS]]  # (b, t, y)
    nc.sync.dma_start(s1g[0:8, :], AP(io["cc_out"], goff, dram_pat))
    nc.sync.dma_start(s1g[BOFF:BOFF + 8, :], AP(io["cc_out"], goff, dram_pat))
    nc.sync.dma_start(s1r[0:8, :], AP(io["cc_out"], roff, dram_pat))
    nc.sync.dma_start(s1r[BOFF:BOFF + 8, :], AP(io["cc_out"], roff, dram_pat))

    emit_sb = vit.tile([J, T * NTAGS], F32, tag="emit")
    nc.vector.memset(emit_sb[:], 0.0)

    def rev_t(ap2d, nrows):
        # view [rows, t, y] with t reversed
        return AP(ap2d.tensor, ap2d.offset + (T - 1) * NTAGS,
                  [list(ap2d.ap[0]), [-NTAGS, T], [1, NTAGS]])

    v3 = lambda a: a.rearrange("p (t y) -> p t y", y=NTAGS)
    nc.vector.tensor_add(v3(emit_sb[0:8, :]), v3(s1g[0:8, :]), rev_t(s1r[0:8, :], 8))
    nc.vector.tensor_add(v3(emit_sb[BOFF:BOFF + 8, :]), rev_t(s1g[BOFF:BOFF + 8, :], 8), v3(s1r[BOFF:BOFF + 8, :]))

    # ---------------- phase 7: the two max-plus scans ----------------
    fv_all = vit.tile([J, (T + 1) * NTAGS], F32, tag="fv_all")
    nc.vector.memset(fv_all[:], NEG)
    nc.sync.dma_start(fv_all[:, 0:NTAGS], io["init"].ap())

    CH = 32  # A-table chunk length (t steps)
    with tc.tile_pool(name="scanA", bufs=2) as apool, \
         tc.tile_pool(name="scan", bufs=3) as scan_pool:
        for c0_ in range(0, T, CH):
            # A[j, t, next, prev] = trans_dual[j, next, prev] + emit[j, t, next]
            A = apool.tile([J, CH * 144], F32, tag="A")
            nc.gpsimd.tensor_tensor(
                out=_ap(A[:], 0, [[144, CH], [NTAGS, NTAGS], [1, NTAGS]]),
                in0=_ap(trans_sb[:], 0, [[0, CH], [NTAGS, NTAGS], [1, NTAGS]]),
                in1=_ap(emit_sb[:], c0_ * NTAGS, [[NTAGS, CH], [1, NTAGS], [0, NTAGS]]),
                op=ALU.add)
            for tt in range(CH):
                t = c0_ + tt
                m_t = scan_pool.tile([J, 144], F32, tag="m")
                fv_b = _ap(fv_all[:], t * NTAGS, [[0, NTAGS], [1, NTAGS]])
                nc.vector.tensor_add(
                    m_t[:].rearrange("p (a b) -> p a b", b=NTAGS), fv_b,
                    _ap(A[:], tt * 144, [[NTAGS, NTAGS], [1, NTAGS]]))
                nc.vector.reduce_max(fv_all[:, (t + 1) * NTAGS:(t + 2) * NTAGS],
                                     m_t[:].rearrange("p (a b) -> p a b", b=NTAGS),
                                     axis=mybir.AxisListType.X)

    if upto < 8:
        nc.gpsimd.dma_start(AP(io["paths"], 0, [[1, 64]]), fv_all[0:1, 0:64])
        return
    # ---------------- phase 8: finalize ----------------
    u2 = vit.tile([BC, T * NTAGS], F32, tag="u2")
    nc.sync.dma_start(u2[:], fv_all[BOFF:BOFF + 8, NTAGS:(T + 1) * NTAGS])

    tA = vit.tile([BC, T * NTAGS], F32, tag="tA")
    nc.vector.tensor_sub(tA[:], fv_all[0:8, NTAGS:(T + 1) * NTAGS], emit_sb[0:8, :])
    tot = vit.tile([BC, T * NTAGS], F32, tag="tot")
    nc.vector.tensor_add(v3(tot[:]), v3(tA[:]), rev_t(u2[:], 8))

    maxv = vit.tile([BC, T], F32, tag="maxv")
    nc.vector.reduce_max(maxv[:], v3(tot[:]), axis=mybir.AxisListType.X)

    eq = vit.tile([BC, T * NTAGS], F32, tag="s1g")
    nc.vector.tensor_tensor(
        out=v3(eq[:]), in0=v3(tot[:]),
        in1=_ap(maxv[:], 0, [[1, T], [0, NTAGS]]),
        op=ALU.is_ge)
    masked = vit.tile([BC, T * NTAGS], F32, tag="s1r")
    nc.vector.scalar_tensor_tensor(
        out=masked[:], in0=eq[:], scalar=1.0e6, in1=iota_f[:],
        op0=ALU.mult, op1=ALU.subtract)
    pm = vit.tile([BC, T], F32, tag="pm")
    nc.vector.reduce_max(pm[:], v3(masked[:]), axis=mybir.AxisListType.X)
    pathf = vit.tile([BC, T], F32, tag="pathf")
    nc.vector.tensor_scalar(out=pathf[:], in0=pm[:], scalar1=-1.0, scalar2=1.0e6,
                            op0=ALU.mult, op1=ALU.add)
    paths_sb = vit.tile([BC, T], I32, tag="paths")
    nc.vector.tensor_copy(paths_sb[:], pathf[:])
    scores_sb = vit.tile([BC, 1], F32, tag="scores")
    nc.vector.tensor_copy(scores_sb[:], maxv[:, 0:1])

    nc.sync.dma_start(io["paths"].ap(), paths_sb[:])
    nc.sync.dma_start(AP(io["scores"], 0, [[1, BC], [1, 1]]), scores_sb[:])


# ---------------------------------------------------------------------------
# host side
# ---------------------------------------------------------------------------

def _wT_blocks(w):
    """[4HH, K] weight -> [128, (ktiles*mtiles)*128] lhsT block layout."""
    kk = w.shape[1] // 128
    mm = w.shape[0] // 128
    wt = w.T.reshape(kk, 128, mm, 128)          # [k, r, m, c]
    return np.ascontiguousarray(wt.transpose(1, 0, 2, 3).reshape(128, kk * mm * 128))


def _make_inputs(inputs):
    sent = np.asarray(inputs["sentence"]).astype(np.int32)
    embed = np.asarray(inputs["embed"]).astype(np.float32)
    h0 = np.asarray(inputs["h0"]).astype(np.float32)
    c0 = np.asarray(inputs["c0"]).astype(np.float32)
    trans = np.asarray(inputs["transitions"]).astype(np.float32)
    b_out = np.asarray(inputs["b_out"]).astype(np.float32)
    W_out = np.asarray(inputs["W_out"]).astype(np.float32)

    Wih = [np.asarray(inputs["W_ih_f"]), np.asarray(inputs["W_ih_b"])]
    Whh = [np.asarray(inputs["W_hh_f"]), np.asarray(inputs["W_hh_b"])]
    bb = [np.asarray(inputs["b_f"]), np.asarray(inputs["b_b"])]

    init_fv = np.full((J, NTAGS), NEG, np.float32)
    init_fv[0:8, START_IDX] = 0.0
    init_fv[32:40, STOP_IDX] = 0.0
    trans_dual = np.tile(trans.reshape(1, 144), (J, 1)).astype(np.float32)
    trans_dual[32:40] = np.tile(trans.T.reshape(1, 144), (8, 1))

    in_maps = []
    for c in range(NCORES):
        d = 1 if c >= 4 else 0
        p = c % 4
        sl = slice(8 * p, 8 * p + 8)
        s = sent[sl]
        if d == 1:
            s = s[:, ::-1]
        flat = np.ascontiguousarray(s.T).reshape(TOK)          # t-major, b-minor
        idx = np.ascontiguousarray(flat.reshape(16, 128).T).astype(np.int32)

        # tanh-trick scaling: i,f,o rows x1/2 (sigmoid via tanh); and the
        # device carries h' = 2h, v = 2c -> W_hh cols x1/2, W_out x1/2, h0 x2, c0 x2.
        rs = np.full((1024, 1), 0.5, np.float32)
        rs[512:768] = 1.0  # g rows keep full scale (plain tanh)
        wih = Wih[d][GATE_PERM] * rs
        whh = Whh[d][GATE_PERM] * rs * 0.5
        bvec = bb[d][GATE_PERM] * rs[:, 0]
        h0s = h0[d][sl] * 2.0   # [8, HH]
        c0s = c0[d][sl] * 2.0
        h0T = np.ascontiguousarray(h0s.T.reshape(KH, 128, 8).transpose(1, 0, 2)
                                   .reshape(128, 16))
        c0T = np.ascontiguousarray(c0s.T.reshape(KH, 128, 8).transpose(1, 0, 2)
                                   .reshape(128, 16))
        wout = W_out[:, d * HH:(d + 1) * HH] * 0.5  # [12, 256]; x1/2 since h'=2h
        woutT = np.ascontiguousarray(wout.T.reshape(KH, 128, NTAGS)
                                     .transpose(1, 0, 2).reshape(128, KH * NTAGS))
        bout_rep = (np.tile(b_out, (128, 1)) if d == 0
                    else np.zeros((128, NTAGS))).astype(np.float32)

        in_maps.append({
            "embed": embed,
            "idx": idx,
            "wihT": _wT_blocks(wih).astype(ml_dtypes.bfloat16),
            "whhT": _wT_blocks(whh).astype(ml_dtypes.bfloat16),
            "woutT": woutT.astype(ml_dtypes.bfloat16),
            "bT": np.ascontiguousarray(bvec.reshape(M, 128).T).astype(np.float32),
            "bout_rep": bout_rep,
            "h0T": h0T.astype(ml_dtypes.bfloat16),
            "c0T": c0T.astype(np.float32),
            "trans_dual": trans_dual,
            "init_fv": init_fv,
        })
    return in_maps


_NC_CACHE = None


def _get_nc():
    global _NC_CACHE
    if _NC_CACHE is None:
        _NC_CACHE = build_program()
    return _NC_CACHE


def kernel(**inputs):
    nc = _get_nc()
    in_maps = _make_inputs(inputs)
    res = run_bass_kernel_spmd(nc, in_maps, core_ids=list(range(NCORES)))
    scores = np.zeros(B, np.float32)
    paths = np.zeros((B, T), np.int32)
    for p in range(4):
        r = res.results[p]
        scores[8 * p:8 * p + 8] = r["out_scores"]
        paths[8 * p:8 * p + 8] = r["out_paths"]
    return scores, paths


if __name__ == "__main__":
    nc = _get_nc()
    print("program built + compiled OK")


# revision 34
# speedup vs baseline: 1.0457x; 1.0099x over previous
"""BiLSTM-CRF (Viterbi decode) Trainium2 Bass kernel.

Sharding: direction-split x batch-split over 8 cores.
  - Pair p in {0,1,2,3}: cores p (forward LSTM) and p+4 (backward LSTM, fed
    time-reversed tokens) both own sentences [8p, 8p+8).
  - Each core: gathers embeddings on-device (indirect DMA), precomputes the
    input contribution Zx = W_ih @ x_t for all t (bf16 matmul), runs the
    sequential LSTM recurrence with stationary bf16 weights (gates land
    transposed: [gate_dim on partitions, batch on free] -> no per-step
    transpose), projects to CRF emission scores, and AllGathers the partial
    emissions within its pair.
  - Viterbi runs WITHOUT traceback: a forward max-plus scan and a backward
    max-plus scan (16 jobs in one set of [16, 144] DVE ops), then
    path[t] = argmax_y(fwd[t,y] + bwd[T-1-t,y] - emit[t,y]) fully in parallel.
    Both cores of a pair redundantly compute all 8 sentences; host reads
    outputs from cores 0-3.
"""

import os
import sys

for _p in ("/opt/trn_rl_repo", "/root/.axon_site/_ro/trn_rl_repo"):
    if os.path.isdir(_p) and _p not in sys.path:
        sys.path.append(_p)

import numpy as np
import ml_dtypes

import concourse.bass as bass
import concourse.tile as tile
from concourse import bacc, mybir
from concourse.bass import AP, IndirectOffsetOnAxis
from concourse.bass_utils import run_bass_kernel_spmd
from concourse.masks import make_identity

F32 = mybir.dt.float32
BF16 = mybir.dt.bfloat16
I32 = mybir.dt.int32
AF = mybir.ActivationFunctionType
ALU = mybir.AluOpType

VOCAB = 100000
EMB = 256
HID = 512
HH = 256  # per-direction hidden
NTAGS = 12
START_IDX = 10
STOP_IDX = 11
NEG = -10000.0

B = 32
T = 256
NCORES = 8
BC = 8          # sentences per pair/core
TOK = T * BC    # 2048 tokens per core
KE = 2          # emb k-tiles
KH = 2          # hidden k-tiles
M = 8           # gate m-tiles (4*HH/128)
J = 40          # viterbi scan partition span: fwd jobs rows 0-7, bwd jobs rows 32-39
BOFF = 32       # partition offset of backward jobs (engine APs need 32-aligned starts)
HSLOT = T + 1   # h history slots (slot 0 = h0)
HFREE = KH * HSLOT * 8  # h_allT free size

# gate order stays (i, f, g, o); with the tanh-trick no contiguity is needed
GATE_PERM = np.arange(1024)

REPLICA_GROUPS = [[0, 4], [1, 5], [2, 6], [3, 7]]


def _ap(t_ap, offset, pattern):
    """New AP over the same tensor with explicit free pattern (keeps partition dim)."""
    return AP(t_ap.tensor, offset, [list(t_ap.ap[0])] + [list(p) for p in pattern])


def build_program(fake_cc=False, num_devices=NCORES, upto=99):
    nc = bacc.Bacc("TRN2", target_bir_lowering=False, debug=False,
                   enable_asserts=False, num_devices=num_devices)

    # ---- I/O ----
    embed_d = nc.dram_tensor("embed", [VOCAB, EMB], F32, kind="ExternalInput")
    idx_d = nc.dram_tensor("idx", [128, 16], I32, kind="ExternalInput")
    wihT_d = nc.dram_tensor("wihT", [128, 2048], BF16, kind="ExternalInput")
    whhT_d = nc.dram_tensor("whhT", [128, 2048], BF16, kind="ExternalInput")
    woutT_d = nc.dram_tensor("woutT", [128, 24], BF16, kind="ExternalInput")
    bT_d = nc.dram_tensor("bT", [128, M], F32, kind="ExternalInput")
    bout_d = nc.dram_tensor("bout_rep", [128, NTAGS], F32, kind="ExternalInput")
    h0T_d = nc.dram_tensor("h0T", [128, 16], BF16, kind="ExternalInput")
    c0T_d = nc.dram_tensor("c0T", [128, 16], F32, kind="ExternalInput")
    trans_d = nc.dram_tensor("trans_dual", [J, 144], F32, kind="ExternalInput")
    init_d = nc.dram_tensor("init_fv", [J, NTAGS], F32, kind="ExternalInput")

    scores_d = nc.dram_tensor("out_scores", [BC], F32, kind="ExternalOutput")
    paths_d = nc.dram_tensor("out_paths", [BC, T], I32, kind="ExternalOutput")

    # collective bounce buffers
    cc_in = nc.dram_tensor("cc_in", [TOK * NTAGS], F32, kind="Internal")
    cc_out = nc.dram_tensor("cc_out", [2 * TOK * NTAGS], F32, kind="Internal")

    from contextlib import ExitStack
    with tile.TileContext(nc) as tc, ExitStack() as ctx:
        _emit(ctx, tc, dict(embed=embed_d, idx=idx_d, wihT=wihT_d, whhT=whhT_d,
                            woutT=woutT_d, bT=bT_d, bout=bout_d, h0T=h0T_d,
                            c0T=c0T_d, trans=trans_d, init=init_d,
                            scores=scores_d, paths=paths_d,
                            cc_in=cc_in, cc_out=cc_out), fake_cc=fake_cc, upto=upto)
    nc.compile()
    return nc


def _emit(ctx, tc, io, fake_cc=False, upto=99):
    nc = tc.nc

    const = ctx.enter_context(tc.tile_pool(name="const", bufs=1))

    ident = const.tile([128, 128], F32, tag="ident")
    make_identity(nc, ident[:])

    # iota over y, [8, T, NTAGS] (values 0..11 are exact in f32)
    iota_f = const.tile([BC, T * NTAGS], F32, tag="iota_f")
    nc.gpsimd.iota(iota_f[:].rearrange("p (t y) -> p t y", y=NTAGS),
                   pattern=[[0, T], [1, NTAGS]], base=0, channel_multiplier=0,
                   allow_small_or_imprecise_dtypes=True)

    # ---- load params ----
    wihT = const.tile([128, 2048], BF16, tag="wihT")
    nc.sync.dma_start(wihT[:], io["wihT"].ap())
    whhT = const.tile([128, 2048], BF16, tag="whhT")
    nc.sync.dma_start(whhT[:], io["whhT"].ap())
    woutT = const.tile([128, 24], BF16, tag="woutT")
    nc.sync.dma_start(woutT[:], io["woutT"].ap())
    bT = const.tile([128, M], F32, tag="bT")
    nc.sync.dma_start(bT[:], io["bT"].ap())
    bout = const.tile([128, NTAGS], F32, tag="bout")
    nc.sync.dma_start(bout[:], io["bout"].ap())
    idx_sb = const.tile([128, 16], I32, tag="idx")
    nc.sync.dma_start(idx_sb[:], io["idx"].ap())
    trans_sb = const.tile([J, 144], F32, tag="trans")
    nc.sync.dma_start(trans_sb[:], io["trans"].ap())
    c0T = const.tile([128, 16], F32, tag="c0T")
    nc.sync.dma_start(c0T[:], io["c0T"].ap())

    # h history: [128, KH * HSLOT * 8] bf16; slot 0 = h0
    hall_pool = ctx.enter_context(tc.tile_pool(name="hall", bufs=1))
    h_allT = hall_pool.tile([128, HFREE], BF16, tag="h_allT")
    nc.sync.dma_start(
        _ap(h_allT[:], 0, [[HSLOT * 8, KH], [1, 8]]),
        AP(io["h0T"], 0, [[16, 128], [8, KH], [1, 8]]))

    # ---------------- phases 1-4 (pools released before Viterbi) ----------------
    # All of gather/transpose/Zx is split into 4 token-chunks (512 tokens = 64
    # steps each) held in separate tiles, so the recurrence can start as soon
    # as chunk 0 is ready instead of waiting for the whole serial prefix.
    NCHUNK, CTOK = 4, 512
    zx_ctx = tc.tile_pool(name="zx", bufs=1)
    zx_pool = zx_ctx.__enter__()
    zxs = [zx_pool.tile([128, M * CTOK], F32, tag=f"zx{n}", name=f"zxc{n}")
           for n in range(NCHUNK)]
    with tc.tile_pool(name="xpool", bufs=1) as xpool, \
         tc.tile_pool(name="xt", bufs=1) as xtpool, \
         tc.tile_pool(name="tp_ps", bufs=4, space="PSUM") as tp_ps, \
         tc.tile_pool(name="zx_ps", bufs=4, space="PSUM") as zx_ps:
        for n in range(NCHUNK):
            xr = xpool.tile([128, 4 * EMB], F32, tag=f"xr{n}")
            for jj in range(4):
                j = n * 4 + jj
                nc.gpsimd.indirect_dma_start(
                    out=xr[:, jj * EMB:(jj + 1) * EMB],
                    out_offset=None,
                    in_=io["embed"].ap(),
                    in_offset=IndirectOffsetOnAxis(ap=idx_sb[:, j:j + 1], axis=0))
            xt = xtpool.tile([128, KE * CTOK], BF16, tag=f"xt{n}")
            for jj in range(4):
                for h in range(KE):
                    ps = tp_ps.tile([128, 128], F32, tag="tp")
                    nc.tensor.transpose(
                        out=ps[:], in_=xr[:, jj * EMB + h * 128: jj * EMB + (h + 1) * 128],
                        identity=ident[:])
                    nc.vector.tensor_copy(
                        xt[:, h * CTOK + jj * 128: h * CTOK + (jj + 1) * 128], ps[:])
            if upto < 3:
                continue
            for m in range(M):
                ps = zx_ps.tile([128, 512], F32, tag="zx")
                for k in range(KE):
                    nc.tensor.matmul(
                        out=ps[:],
                        lhsT=wihT[:, (k * M + m) * 128:(k * M + m + 1) * 128],
                        rhs=xt[:, k * CTOK:(k + 1) * CTOK],
                        start=(k == 0), stop=(k == KE - 1))
                dst = zxs[n][:, m * CTOK:(m + 1) * CTOK]
                if (m + n) % 2 == 0:
                    nc.scalar.activation(out=dst, in_=ps[:], func=AF.Identity,
                                         bias=bT[:, m:m + 1], scale=1.0)
                else:
                    nc.vector.tensor_scalar_add(dst, ps[:], bT[:, m:m + 1])
        if upto < 3:
            nc.gpsimd.dma_start(AP(io["paths"], 0, [[1, 64]]), xt[0:1, 0:64])
            return

    if upto < 4:
        nc.gpsimd.dma_start(AP(io["paths"], 0, [[1, 64]]), zxs[0][0:1, 0:64])
        return
    # ---------------- phase 4: LSTM recurrence ----------------
    rec_ctxs = [tc.tile_pool(name="rec_ps", bufs=4, space="PSUM"),
                tc.tile_pool(name="gates", bufs=4),
                tc.tile_pool(name="small", bufs=8),
                tc.tile_pool(name="cpool", bufs=4)]
    rec_ps, gates, small, cpool = [c.__enter__() for c in rec_ctxs]

    c_prev = cpool.tile([128, 16], F32, tag="c")
    nc.vector.tensor_copy(c_prev[:], c0T[:])

    zx_views = [z[:].rearrange("p (m t b) -> p m t b", m=M, b=BC)
                for z in zxs]
    for t in range(T):
        ps = rec_ps.tile([128, M * BC], F32, tag="rec")
        for m in range(M):
            for k in range(KH):
                nc.tensor.matmul(
                    out=ps[:, m * BC:(m + 1) * BC],
                    lhsT=whhT[:, (k * M + m) * 128:(k * M + m + 1) * 128],
                    rhs=h_allT[:, k * HSLOT * 8 + t * 8: k * HSLOT * 8 + (t + 1) * 8],
                    start=(k == 0), stop=(k == KH - 1))
        # gates via ONE tanh per half: i,f,o rows pre-scaled by 1/2 host-side, so
        # tanh(z/2) -> sigmoid(z) = (th+1)/2; g is a plain tanh. The cell state is
        # kept as v = 2c and hidden as h' = 2h (weights pre-scaled), which turns
        # every product into a fused (x+1)*y / (0.5x+y) scalar_tensor_tensor op.
        # The o-gate half (m-tiles 6,7) is processed separately so its add+tanh
        # overlaps the i/f/g -> v -> tanh(c) chain.
        gp = gates.tile([128, 48], F32, tag="gp")
        nc.vector.tensor_add(
            gp[:, 0:48].rearrange("p (m b) -> p m b", b=BC),
            ps[:, 0:48].rearrange("p (m b) -> p m b", b=BC),
            zx_views[t // 64][:, 0:6, t % 64, :])
        th = gates.tile([128, 48], F32, tag="th")
        nc.scalar.activation(out=th[:, 0:48], in_=gp[:, 0:48], func=AF.Tanh)
        u1 = small.tile([128, 16], F32, tag="u1")
        nc.vector.scalar_tensor_tensor(out=u1[:], in0=th[:, 16:32], scalar=1.0,
                                       in1=c_prev[:], op0=ALU.add, op1=ALU.mult)
        u2 = small.tile([128, 16], F32, tag="u2")
        nc.vector.scalar_tensor_tensor(out=u2[:], in0=th[:, 0:16], scalar=1.0,
                                       in1=th[:, 32:48], op0=ALU.add, op1=ALU.mult)
        c_new = cpool.tile([128, 16], F32, tag="c")
        nc.vector.scalar_tensor_tensor(out=c_new[:], in0=u1[:], scalar=0.5,
                                       in1=u2[:], op0=ALU.mult, op1=ALU.add)
        tc_t = small.tile([128, 16], F32, tag="tc")
        nc.scalar.activation(out=tc_t[:], in_=c_new[:], func=AF.Tanh, scale=0.5)
        gp2 = small.tile([128, 16], F32, tag="gp2")
        nc.vector.tensor_add(
            gp2[:].rearrange("p (m b) -> p m b", b=BC),
            ps[:, 48:64].rearrange("p (m b) -> p m b", b=BC),
            zx_views[t // 64][:, 6:8, t % 64, :])
        th2 = small.tile([128, 16], F32, tag="th2")
        nc.scalar.activation(out=th2[:], in_=gp2[:], func=AF.Tanh)
        hout = _ap(h_allT[:], (t + 1) * 8, [[HSLOT * 8, KH], [1, 8]])
        nc.vector.scalar_tensor_tensor(
            out=hout,
            in0=th2[:].rearrange("p (k b) -> p k b", b=8), scalar=1.0,
            in1=tc_t[:].rearrange("p (k b) -> p k b", b=8),
            op0=ALU.add, op1=ALU.mult)
        c_prev = c_new

    for c in reversed(rec_ctxs):
        c.__exit__(None, None, None)
    zx_ctx.__exit__(None, None, None)

    if upto < 5:
        nc.gpsimd.dma_start(AP(io["paths"], 0, [[1, 64]]), h_allT[0:1, 0:128].bitcast(I32))
        return
    # ---------------- phase 5: emission partials + AllGather ----------------
    vit = ctx.enter_context(tc.tile_pool(name="vit", bufs=1))
    feats_sb = vit.tile([128, 16 * NTAGS], F32, tag="feats")
    with tc.tile_pool(name="f_ps", bufs=4, space="PSUM") as f_ps:
        for mc in range(16):
            ps = f_ps.tile([128, NTAGS], F32, tag="f")
            for k in range(KH):
                nc.tensor.matmul(
                    out=ps[:],
                    lhsT=h_allT[:, k * HSLOT * 8 + 8 + mc * 128: k * HSLOT * 8 + 8 + (mc + 1) * 128],
                    rhs=woutT[:, k * NTAGS:(k + 1) * NTAGS],
                    start=(k == 0), stop=(k == KH - 1))
            nc.vector.tensor_add(feats_sb[:, mc * NTAGS:(mc + 1) * NTAGS],
                                 ps[:], bout[:])

    # SBUF [128 p, (chunk 16, y 12)] -> DRAM flat [(chunk*128+p) * 12 + y]
    nc.sync.dma_start(
        AP(io["cc_in"], 0, [[NTAGS, 128], [128 * NTAGS, 16], [1, NTAGS]]),
        feats_sb[:].rearrange("p (c y) -> p c y", y=NTAGS))
    if fake_cc:
        # single-core timing-model variant: stand in for the pair AllGather
        half = TOK * NTAGS
        nc.sync.dma_start(AP(io["cc_out"], 0, [[1, half]]), io["cc_in"].ap())
        nc.sync.dma_start(AP(io["cc_out"], half, [[1, half]]), io["cc_in"].ap())
    else:
        nc.gpsimd.collective_compute(
            "AllGather", ALU.bypass, replica_groups=REPLICA_GROUPS,
            ins=[io["cc_in"].ap()], outs=[io["cc_out"].ap()])

    # ---------------- phase 6: build emit_sb [J, T*12] ----------------
    # G = rank0 partial (global t order), R = rank1 partial (reversed t order)
    s1g = vit.tile([J, T * NTAGS], F32, tag="s1g")
    s1r = vit.tile([J, T * NTAGS], F32, tag="s1r")
    goff, roff = 0, TOK * NTAGS
    dram_pat = [[NTAGS, BC], [BC * NTAGS, T], [1, NTAGS]]  # (b, t, y)
    nc.sync.dma_start(s1g[0:8, :], AP(io["cc_out"], goff, dram_pat))
    nc.sync.dma_start(s1g[BOFF:BOFF + 8, :], AP(io["cc_out"], goff, dram_pat))
    nc.sync.dma_start(s1r[0:8, :], AP(io["cc_out"], roff, dram_pat))
    nc.sync.dma_start(s1r[BOFF:BOFF + 8, :], AP(io["cc_out"], roff, dram_pat))

    emit_sb = vit.tile([J, T * NTAGS], F32, tag="emit")
    nc.vector.memset(emit_sb[:], 0.0)

    def rev_t(ap2d, nrows):
        # view [rows, t, y] with t reversed
        return AP(ap2d.tensor, ap2d.offset + (T - 1) * NTAGS,
                  [list(ap2d.ap[0]), [-NTAGS, T], [1, NTAGS]])

    v3 = lambda a: a.rearrange("p (t y) -> p t y", y=NTAGS)
    nc.vector.tensor_add(v3(emit_sb[0:8, :]), v3(s1g[0:8, :]), rev_t(s1r[0:8, :], 8))
    nc.vector.tensor_add(v3(emit_sb[BOFF:BOFF + 8, :]), rev_t(s1g[BOFF:BOFF + 8, :], 8), v3(s1r[BOFF:BOFF + 8, :]))

    # ---------------- phase 7: the two max-plus scans ----------------
    fv_all = vit.tile([J, (T + 1) * NTAGS], F32, tag="fv_all")
    nc.vector.memset(fv_all[:], NEG)
    nc.sync.dma_start(fv_all[:, 0:NTAGS], io["init"].ap())

    CH = 32  # A-table chunk length (t steps)
    with tc.tile_pool(name="scanA", bufs=2) as apool, \
         tc.tile_pool(name="scan", bufs=3) as scan_pool:
        for c0_ in range(0, T, CH):
            # A[j, t, next, prev] = trans_dual[j, next, prev] + emit[j, t, next]
            A = apool.tile([J, CH * 144], F32, tag="A")
            nc.gpsimd.tensor_tensor(
                out=_ap(A[:], 0, [[144, CH], [NTAGS, NTAGS], [1, NTAGS]]),
                in0=_ap(trans_sb[:], 0, [[0, CH], [NTAGS, NTAGS], [1, NTAGS]]),
                in1=_ap(emit_sb[:], c0_ * NTAGS, [[NTAGS, CH], [1, NTAGS], [0, NTAGS]]),
                op=ALU.add)
            for tt in range(CH):
                t = c0_ + tt
                m_t = scan_pool.tile([J, 144], F32, tag="m")
                fv_b = _ap(fv_all[:], t * NTAGS, [[0, NTAGS], [1, NTAGS]])
                nc.vector.tensor_add(
                    m_t[:].rearrange("p (a b) -> p a b", b=NTAGS), fv_b,
                    _ap(A[:], tt * 144, [[NTAGS, NTAGS], [1, NTAGS]]))
                nc.vector.reduce_max(fv_all[:, (t + 1) * NTAGS:(t + 2) * NTAGS],
                                     m_t[:].rearrange("p (a b) -> p a b", b=NTAGS),
                                     axis=mybir.AxisListType.X)

    if upto < 8:
        nc.gpsimd.dma_start(AP(io["paths"], 0, [[1, 64]]), fv_all[0:1, 0:64])
        return
    # ---------------- phase 8: finalize ----------------
    u2 = vit.tile([BC, T * NTAGS], F32, tag="u2")
    nc.sync.dma_start(u2[:], fv_all[BOFF:BOFF + 8, NTAGS:(T + 1) * NTAGS])

    tA = vit.tile([BC, T * NTAGS], F32, tag="tA")
    nc.vector.tensor_sub(tA[:], fv_all[0:8, NTAGS:(T + 1) * NTAGS], emit_sb[0:8, :])
    tot = vit.tile([BC, T * NTAGS], F32, tag="tot")
    nc.vector.tensor_add(v3(tot[:]), v3(tA[:]), rev_t(u2[:], 8))

    maxv = vit.tile([BC, T], F32, tag="maxv")
    nc.vector.reduce_max(maxv[:], v3(tot[:]), axis=mybir.AxisListType.X)

    eq = vit.tile([BC, T * NTAGS], F32, tag="s1g")
    nc.vector.tensor_tensor(
        out=v3(eq[:]), in0=v3(tot[:]),
        in1=_ap(maxv[:], 0, [[1, T], [0, NTAGS]]),
        op=ALU.is_ge)
    masked = vit.tile([BC, T * NTAGS], F32, tag="s1r")
    nc.vector.scalar_tensor_tensor(
        out=masked[:], in0=eq[:], scalar=1.0e6, in1=iota_f[:],
        op0=ALU.mult, op1=ALU.subtract)
    pm = vit.tile([BC, T], F32, tag="pm")
    nc.vector.reduce_max(pm[:], v3(masked[:]), axis=mybir.AxisListType.X)
    pathf = vit.tile([BC, T], F32, tag="pathf")
    nc.vector.tensor_scalar(out=pathf[:], in0=pm[:], scalar1=-1.0, scalar2=1.0e6,
                            op0=ALU.mult, op1=ALU.add)
    paths_sb = vit.tile([BC, T], I32, tag="paths")
    nc.vector.tensor_copy(paths_sb[:], pathf[:])
    scores_sb = vit.tile([BC, 1], F32, tag="scores")
    nc.vector.tensor_copy(scores_sb[:], maxv[:, 0:1])

    nc.sync.dma_start(io["paths"].ap(), paths_sb[:])
    nc.sync.dma_start(AP(io["scores"], 0, [[1, BC], [1, 1]]), scores_sb[:])


# ---------------------------------------------------------------------------
# host side
# ---------------------------------------------------------------------------

def _wT_blocks(w):
    """[4HH, K] weight -> [128, (ktiles*mtiles)*128] lhsT block layout."""
    kk = w.shape[1] // 128
    mm = w.shape[0] // 128
    wt = w.T.reshape(kk, 128, mm, 128)          # [k, r, m, c]
    return np.ascontiguousarray(wt.transpose(1, 0, 2, 3).reshape(128, kk * mm * 128))


def _make_inputs(inputs):
    sent = np.asarray(inputs["sentence"]).astype(np.int32)
    embed = np.asarray(inputs["embed"]).astype(np.float32)
    h0 = np.asarray(inputs["h0"]).astype(np.float32)
    c0 = np.asarray(inputs["c0"]).astype(np.float32)
    trans = np.asarray(inputs["transitions"]).astype(np.float32)
    b_out = np.asarray(inputs["b_out"]).astype(np.float32)
    W_out = np.asarray(inputs["W_out"]).astype(np.float32)

    Wih = [np.asarray(inputs["W_ih_f"]), np.asarray(inputs["W_ih_b"])]
    Whh = [np.asarray(inputs["W_hh_f"]), np.asarray(inputs["W_hh_b"])]
    bb = [np.asarray(inputs["b_f"]), np.asarray(inputs["b_b"])]

    init_fv = np.full((J, NTAGS), NEG, np.float32)
    init_fv[0:8, START_IDX] = 0.0
    init_fv[32:40, STOP_IDX] = 0.0
    trans_dual = np.tile(trans.reshape(1, 144), (J, 1)).astype(np.float32)
    trans_dual[32:40] = np.tile(trans.T.reshape(1, 144), (8, 1))

    in_maps = []
    for c in range(NCORES):
        d = 1 if c >= 4 else 0
        p = c % 4
        sl = slice(8 * p, 8 * p + 8)
        s = sent[sl]
        if d == 1:
            s = s[:, ::-1]
        flat = np.ascontiguousarray(s.T).reshape(TOK)          # t-major, b-minor
        idx = np.ascontiguousarray(flat.reshape(16, 128).T).astype(np.int32)

        # tanh-trick scaling: i,f,o rows x1/2 (sigmoid via tanh); and the
        # device carries h' = 2h, v = 2c -> W_hh cols x1/2, W_out x1/2, h0 x2, c0 x2.
        rs = np.full((1024, 1), 0.5, np.float32)
        rs[512:768] = 1.0  # g rows keep full scale (plain tanh)
        wih = Wih[d][GATE_PERM] * rs
        whh = Whh[d][GATE_PERM] * rs * 0.5
        bvec = bb[d][GATE_PERM] * rs[:, 0]
        h0s = h0[d][sl] * 2.0   # [8, HH]
        c0s = c0[d][sl] * 2.0
        h0T = np.ascontiguousarray(h0s.T.reshape(KH, 128, 8).transpose(1, 0, 2)
                                   .reshape(128, 16))
        c0T = np.ascontiguousarray(c0s.T.reshape(KH, 128, 8).transpose(1, 0, 2)
                                   .reshape(128, 16))
        wout = W_out[:, d * HH:(d + 1) * HH] * 0.5  # [12, 256]; x1/2 since h'=2h
        woutT = np.ascontiguousarray(wout.T.reshape(KH, 128, NTAGS)
                                     .transpose(1, 0, 2).reshape(128, KH * NTAGS))
        bout_rep = (np.tile(b_out, (128, 1)) if d == 0
                    else np.zeros((128, NTAGS))).astype(np.float32)

        in_maps.append({
            "embed": embed,
            "idx": idx,
            "wihT": _wT_blocks(wih).astype(ml_dtypes.bfloat16),
            "whhT": _wT_blocks(whh).astype(ml_dtypes.bfloat16),
            "woutT": woutT.astype(ml_dtypes.bfloat16),
            "bT": np.ascontiguousarray(bvec.reshape(M, 128).T).astype(np.float32),
            "bout_rep": bout_rep,
            "h0T": h0T.astype(ml_dtypes.bfloat16),
            "c0T": c0T.astype(np.float32),
            "trans_dual": trans_dual,
            "init_fv": init_fv,
        })
    return in_maps


_NC_CACHE = None


def _get_nc():
    global _NC_CACHE
    if _NC_CACHE is None:
        _NC_CACHE = build_program()
    return _NC_CACHE


def kernel(**inputs):
    nc = _get_nc()
    in_maps = _make_inputs(inputs)
    res = run_bass_kernel_spmd(nc, in_maps, core_ids=list(range(NCORES)))
    scores = np.zeros(B, np.float32)
    paths = np.zeros((B, T), np.int32)
    for p in range(4):
        r = res.results[p]
        scores[8 * p:8 * p + 8] = r["out_scores"]
        paths[8 * p:8 * p + 8] = r["out_paths"]
    return scores, paths


if __name__ == "__main__":
    nc = _get_nc()
    print("program built + compiled OK")


# revision 35
# speedup vs baseline: 1.0466x; 1.0008x over previous
"""BiLSTM-CRF (Viterbi decode) Trainium2 Bass kernel.

Sharding: direction-split x batch-split over 8 cores.
  - Pair p in {0,1,2,3}: cores p (forward LSTM) and p+4 (backward LSTM, fed
    time-reversed tokens) both own sentences [8p, 8p+8).
  - Each core: gathers embeddings on-device (indirect DMA), precomputes the
    input contribution Zx = W_ih @ x_t for all t (bf16 matmul), runs the
    sequential LSTM recurrence with stationary bf16 weights (gates land
    transposed: [gate_dim on partitions, batch on free] -> no per-step
    transpose), projects to CRF emission scores, and AllGathers the partial
    emissions within its pair.
  - Viterbi runs WITHOUT traceback: a forward max-plus scan and a backward
    max-plus scan (16 jobs in one set of [16, 144] DVE ops), then
    path[t] = argmax_y(fwd[t,y] + bwd[T-1-t,y] - emit[t,y]) fully in parallel.
    Both cores of a pair redundantly compute all 8 sentences; host reads
    outputs from cores 0-3.
"""

import os
import sys

for _p in ("/opt/trn_rl_repo", "/root/.axon_site/_ro/trn_rl_repo"):
    if os.path.isdir(_p) and _p not in sys.path:
        sys.path.append(_p)

import numpy as np
import ml_dtypes

import concourse.bass as bass
import concourse.tile as tile
from concourse import bacc, mybir
from concourse.bass import AP, IndirectOffsetOnAxis
from concourse.bass_utils import run_bass_kernel_spmd
from concourse.masks import make_identity

F32 = mybir.dt.float32
BF16 = mybir.dt.bfloat16
I32 = mybir.dt.int32
AF = mybir.ActivationFunctionType
ALU = mybir.AluOpType

VOCAB = 100000
EMB = 256
HID = 512
HH = 256  # per-direction hidden
NTAGS = 12
START_IDX = 10
STOP_IDX = 11
NEG = -10000.0

B = 32
T = 256
NCORES = 8
BC = 8          # sentences per pair/core
TOK = T * BC    # 2048 tokens per core
KE = 2          # emb k-tiles
KH = 2          # hidden k-tiles
M = 8           # gate m-tiles (4*HH/128)
J = 40          # viterbi scan partition span: fwd jobs rows 0-7, bwd jobs rows 32-39
BOFF = 32       # partition offset of backward jobs (engine APs need 32-aligned starts)
HSLOT = T + 1   # h history slots (slot 0 = h0)
HFREE = KH * HSLOT * 8  # h_allT free size

# gate order stays (i, f, g, o); with the tanh-trick no contiguity is needed
GATE_PERM = np.arange(1024)

REPLICA_GROUPS = [[0, 4], [1, 5], [2, 6], [3, 7]]


def _ap(t_ap, offset, pattern):
    """New AP over the same tensor with explicit free pattern (keeps partition dim)."""
    return AP(t_ap.tensor, offset, [list(t_ap.ap[0])] + [list(p) for p in pattern])


def build_program(fake_cc=False, num_devices=NCORES, upto=99):
    nc = bacc.Bacc("TRN2", target_bir_lowering=False, debug=False,
                   enable_asserts=False, num_devices=num_devices)

    # ---- I/O ----
    embed_d = nc.dram_tensor("embed", [VOCAB, EMB], F32, kind="ExternalInput")
    idx_d = nc.dram_tensor("idx", [128, 16], I32, kind="ExternalInput")
    wihT_d = nc.dram_tensor("wihT", [128, 2048], BF16, kind="ExternalInput")
    whhT_d = nc.dram_tensor("whhT", [128, 2048], BF16, kind="ExternalInput")
    woutT_d = nc.dram_tensor("woutT", [128, 24], BF16, kind="ExternalInput")
    bT_d = nc.dram_tensor("bT", [128, M], F32, kind="ExternalInput")
    bout_d = nc.dram_tensor("bout_rep", [128, NTAGS], F32, kind="ExternalInput")
    h0T_d = nc.dram_tensor("h0T", [128, 16], BF16, kind="ExternalInput")
    c0T_d = nc.dram_tensor("c0T", [128, 16], F32, kind="ExternalInput")
    trans_d = nc.dram_tensor("trans_dual", [J, 144], F32, kind="ExternalInput")
    init_d = nc.dram_tensor("init_fv", [J, NTAGS], F32, kind="ExternalInput")

    scores_d = nc.dram_tensor("out_scores", [BC], F32, kind="ExternalOutput")
    paths_d = nc.dram_tensor("out_paths", [BC, T], I32, kind="ExternalOutput")

    # collective bounce buffers
    cc_in = nc.dram_tensor("cc_in", [TOK * NTAGS], F32, kind="Internal")
    cc_out = nc.dram_tensor("cc_out", [2 * TOK * NTAGS], F32, kind="Internal")

    from contextlib import ExitStack
    with tile.TileContext(nc) as tc, ExitStack() as ctx:
        _emit(ctx, tc, dict(embed=embed_d, idx=idx_d, wihT=wihT_d, whhT=whhT_d,
                            woutT=woutT_d, bT=bT_d, bout=bout_d, h0T=h0T_d,
                            c0T=c0T_d, trans=trans_d, init=init_d,
                            scores=scores_d, paths=paths_d,
                            cc_in=cc_in, cc_out=cc_out), fake_cc=fake_cc, upto=upto)
    nc.compile()
    return nc


def _emit(ctx, tc, io, fake_cc=False, upto=99):
    nc = tc.nc

    const = ctx.enter_context(tc.tile_pool(name="const", bufs=1))

    ident = const.tile([128, 128], F32, tag="ident")
    make_identity(nc, ident[:])

    # iota over y, [8, T, NTAGS] (values 0..11 are exact in f32)
    iota_f = const.tile([BC, T * NTAGS], F32, tag="iota_f")
    nc.gpsimd.iota(iota_f[:].rearrange("p (t y) -> p t y", y=NTAGS),
                   pattern=[[0, T], [1, NTAGS]], base=0, channel_multiplier=0,
                   allow_small_or_imprecise_dtypes=True)

    # ---- load params ----
    wihT = const.tile([128, 2048], BF16, tag="wihT")
    nc.sync.dma_start(wihT[:], io["wihT"].ap())
    whhT = const.tile([128, 2048], BF16, tag="whhT")
    nc.sync.dma_start(whhT[:], io["whhT"].ap())
    woutT = const.tile([128, 24], BF16, tag="woutT")
    nc.sync.dma_start(woutT[:], io["woutT"].ap())
    bT = const.tile([128, M], F32, tag="bT")
    nc.sync.dma_start(bT[:], io["bT"].ap())
    bout = const.tile([128, NTAGS], F32, tag="bout")
    nc.sync.dma_start(bout[:], io["bout"].ap())
    idx_sb = const.tile([128, 16], I32, tag="idx")
    nc.sync.dma_start(idx_sb[:], io["idx"].ap())
    trans_sb = const.tile([J, 144], F32, tag="trans")
    nc.sync.dma_start(trans_sb[:], io["trans"].ap())
    c0T = const.tile([128, 16], F32, tag="c0T")
    nc.sync.dma_start(c0T[:], io["c0T"].ap())

    # h history: [128, KH * HSLOT * 8] bf16; slot 0 = h0
    hall_pool = ctx.enter_context(tc.tile_pool(name="hall", bufs=1))
    h_allT = hall_pool.tile([128, HFREE], BF16, tag="h_allT")
    nc.sync.dma_start(
        _ap(h_allT[:], 0, [[HSLOT * 8, KH], [1, 8]]),
        AP(io["h0T"], 0, [[16, 128], [8, KH], [1, 8]]))

    # emit/fv live in a pre-recurrence pool whose addresses never overlap the
    # zx/x pools, so their big memsets + init DMA run during the gather phase
    # instead of serializing at the recurrence->Viterbi junction.
    early = ctx.enter_context(tc.tile_pool(name="vearly", bufs=1))
    emit_sb = early.tile([J, T * NTAGS], F32, tag="emit")
    nc.vector.memset(emit_sb[:], 0.0)
    fv_all = early.tile([J, (T + 1) * NTAGS], F32, tag="fv_all")
    nc.vector.memset(fv_all[:], NEG)
    nc.sync.dma_start(fv_all[:, 0:NTAGS], io["init"].ap())

    # ---------------- phases 1-4 (pools released before Viterbi) ----------------
    # All of gather/transpose/Zx is split into 4 token-chunks (512 tokens = 64
    # steps each) held in separate tiles, so the recurrence can start as soon
    # as chunk 0 is ready instead of waiting for the whole serial prefix.
    NCHUNK, CTOK = 4, 512
    zx_ctx = tc.tile_pool(name="zx", bufs=1)
    zx_pool = zx_ctx.__enter__()
    zxs = [zx_pool.tile([128, M * CTOK], F32, tag=f"zx{n}", name=f"zxc{n}")
           for n in range(NCHUNK)]
    with tc.tile_pool(name="xpool", bufs=1) as xpool, \
         tc.tile_pool(name="xt", bufs=1) as xtpool, \
         tc.tile_pool(name="tp_ps", bufs=4, space="PSUM") as tp_ps, \
         tc.tile_pool(name="zx_ps", bufs=4, space="PSUM") as zx_ps:
        for n in range(NCHUNK):
            xr = xpool.tile([128, 4 * EMB], F32, tag=f"xr{n}")
            for jj in range(4):
                j = n * 4 + jj
                nc.gpsimd.indirect_dma_start(
                    out=xr[:, jj * EMB:(jj + 1) * EMB],
                    out_offset=None,
                    in_=io["embed"].ap(),
                    in_offset=IndirectOffsetOnAxis(ap=idx_sb[:, j:j + 1], axis=0))
            xt = xtpool.tile([128, KE * CTOK], BF16, tag=f"xt{n}")
            for jj in range(4):
                for h in range(KE):
                    ps = tp_ps.tile([128, 128], F32, tag="tp")
                    nc.tensor.transpose(
                        out=ps[:], in_=xr[:, jj * EMB + h * 128: jj * EMB + (h + 1) * 128],
                        identity=ident[:])
                    nc.vector.tensor_copy(
                        xt[:, h * CTOK + jj * 128: h * CTOK + (jj + 1) * 128], ps[:])
            if upto < 3:
                continue
            for m in range(M):
                ps = zx_ps.tile([128, 512], F32, tag="zx")
                for k in range(KE):
                    nc.tensor.matmul(
                        out=ps[:],
                        lhsT=wihT[:, (k * M + m) * 128:(k * M + m + 1) * 128],
                        rhs=xt[:, k * CTOK:(k + 1) * CTOK],
                        start=(k == 0), stop=(k == KE - 1))
                dst = zxs[n][:, m * CTOK:(m + 1) * CTOK]
                if (m + n) % 2 == 0:
                    nc.scalar.activation(out=dst, in_=ps[:], func=AF.Identity,
                                         bias=bT[:, m:m + 1], scale=1.0)
                else:
                    nc.vector.tensor_scalar_add(dst, ps[:], bT[:, m:m + 1])
        if upto < 3:
            nc.gpsimd.dma_start(AP(io["paths"], 0, [[1, 64]]), xt[0:1, 0:64])
            return

    if upto < 4:
        nc.gpsimd.dma_start(AP(io["paths"], 0, [[1, 64]]), zxs[0][0:1, 0:64])
        return
    # ---------------- phase 4: LSTM recurrence ----------------
    rec_ctxs = [tc.tile_pool(name="rec_ps", bufs=4, space="PSUM"),
                tc.tile_pool(name="gates", bufs=4),
                tc.tile_pool(name="small", bufs=8),
                tc.tile_pool(name="cpool", bufs=4)]
    rec_ps, gates, small, cpool = [c.__enter__() for c in rec_ctxs]

    c_prev = cpool.tile([128, 16], F32, tag="c")
    nc.vector.tensor_copy(c_prev[:], c0T[:])

    zx_views = [z[:].rearrange("p (m t b) -> p m t b", m=M, b=BC)
                for z in zxs]
    for t in range(T):
        ps = rec_ps.tile([128, M * BC], F32, tag="rec")
        for m in range(M):
            for k in range(KH):
                nc.tensor.matmul(
                    out=ps[:, m * BC:(m + 1) * BC],
                    lhsT=whhT[:, (k * M + m) * 128:(k * M + m + 1) * 128],
                    rhs=h_allT[:, k * HSLOT * 8 + t * 8: k * HSLOT * 8 + (t + 1) * 8],
                    start=(k == 0), stop=(k == KH - 1))
        # gates via ONE tanh per half: i,f,o rows pre-scaled by 1/2 host-side, so
        # tanh(z/2) -> sigmoid(z) = (th+1)/2; g is a plain tanh. The cell state is
        # kept as v = 2c and hidden as h' = 2h (weights pre-scaled), which turns
        # every product into a fused (x+1)*y / (0.5x+y) scalar_tensor_tensor op.
        # The o-gate half (m-tiles 6,7) is processed separately so its add+tanh
        # overlaps the i/f/g -> v -> tanh(c) chain.
        gp = gates.tile([128, 48], F32, tag="gp")
        nc.vector.tensor_add(
            gp[:, 0:48].rearrange("p (m b) -> p m b", b=BC),
            ps[:, 0:48].rearrange("p (m b) -> p m b", b=BC),
            zx_views[t // 64][:, 0:6, t % 64, :])
        th = gates.tile([128, 48], F32, tag="th")
        nc.scalar.activation(out=th[:, 0:48], in_=gp[:, 0:48], func=AF.Tanh)
        u1 = small.tile([128, 16], F32, tag="u1")
        nc.vector.scalar_tensor_tensor(out=u1[:], in0=th[:, 16:32], scalar=1.0,
                                       in1=c_prev[:], op0=ALU.add, op1=ALU.mult)
        u2 = small.tile([128, 16], F32, tag="u2")
        nc.vector.scalar_tensor_tensor(out=u2[:], in0=th[:, 0:16], scalar=1.0,
                                       in1=th[:, 32:48], op0=ALU.add, op1=ALU.mult)
        c_new = cpool.tile([128, 16], F32, tag="c")
        nc.vector.scalar_tensor_tensor(out=c_new[:], in0=u1[:], scalar=0.5,
                                       in1=u2[:], op0=ALU.mult, op1=ALU.add)
        tc_t = small.tile([128, 16], F32, tag="tc")
        nc.scalar.activation(out=tc_t[:], in_=c_new[:], func=AF.Tanh, scale=0.5)
        gp2 = small.tile([128, 16], F32, tag="gp2")
        nc.vector.tensor_add(
            gp2[:].rearrange("p (m b) -> p m b", b=BC),
            ps[:, 48:64].rearrange("p (m b) -> p m b", b=BC),
            zx_views[t // 64][:, 6:8, t % 64, :])
        th2 = small.tile([128, 16], F32, tag="th2")
        nc.scalar.activation(out=th2[:], in_=gp2[:], func=AF.Tanh)
        hout = _ap(h_allT[:], (t + 1) * 8, [[HSLOT * 8, KH], [1, 8]])
        nc.vector.scalar_tensor_tensor(
            out=hout,
            in0=th2[:].rearrange("p (k b) -> p k b", b=8), scalar=1.0,
            in1=tc_t[:].rearrange("p (k b) -> p k b", b=8),
            op0=ALU.add, op1=ALU.mult)
        c_prev = c_new

    for c in reversed(rec_ctxs):
        c.__exit__(None, None, None)
    zx_ctx.__exit__(None, None, None)

    if upto < 5:
        nc.gpsimd.dma_start(AP(io["paths"], 0, [[1, 64]]), h_allT[0:1, 0:128].bitcast(I32))
        return
    # ---------------- phase 5: emission partials + AllGather ----------------
    vit = ctx.enter_context(tc.tile_pool(name="vit", bufs=1))
    feats_sb = vit.tile([128, 16 * NTAGS], F32, tag="feats")
    with tc.tile_pool(name="f_ps", bufs=4, space="PSUM") as f_ps:
        for mc in range(16):
            ps = f_ps.tile([128, NTAGS], F32, tag="f")
            for k in range(KH):
                nc.tensor.matmul(
                    out=ps[:],
                    lhsT=h_allT[:, k * HSLOT * 8 + 8 + mc * 128: k * HSLOT * 8 + 8 + (mc + 1) * 128],
                    rhs=woutT[:, k * NTAGS:(k + 1) * NTAGS],
                    start=(k == 0), stop=(k == KH - 1))
            nc.vector.tensor_add(feats_sb[:, mc * NTAGS:(mc + 1) * NTAGS],
                                 ps[:], bout[:])

    # SBUF [128 p, (chunk 16, y 12)] -> DRAM flat [(chunk*128+p) * 12 + y]
    nc.sync.dma_start(
        AP(io["cc_in"], 0, [[NTAGS, 128], [128 * NTAGS, 16], [1, NTAGS]]),
        feats_sb[:].rearrange("p (c y) -> p c y", y=NTAGS))
    if fake_cc:
        # single-core timing-model variant: stand in for the pair AllGather
        half = TOK * NTAGS
        nc.sync.dma_start(AP(io["cc_out"], 0, [[1, half]]), io["cc_in"].ap())
        nc.sync.dma_start(AP(io["cc_out"], half, [[1, half]]), io["cc_in"].ap())
    else:
        nc.gpsimd.collective_compute(
            "AllGather", ALU.bypass, replica_groups=REPLICA_GROUPS,
            ins=[io["cc_in"].ap()], outs=[io["cc_out"].ap()])

    # ---------------- phase 6: build emit_sb [J, T*12] ----------------
    # G = rank0 partial (global t order), R = rank1 partial (reversed t order)
    s1g = vit.tile([J, T * NTAGS], F32, tag="s1g")
    s1r = vit.tile([J, T * NTAGS], F32, tag="s1r")
    goff, roff = 0, TOK * NTAGS
    dram_pat = [[NTAGS, BC], [BC * NTAGS, T], [1, NTAGS]]  # (b, t, y)
    nc.sync.dma_start(s1g[0:8, :], AP(io["cc_out"], goff, dram_pat))
    nc.sync.dma_start(s1g[BOFF:BOFF + 8, :], AP(io["cc_out"], goff, dram_pat))
    nc.sync.dma_start(s1r[0:8, :], AP(io["cc_out"], roff, dram_pat))
    nc.sync.dma_start(s1r[BOFF:BOFF + 8, :], AP(io["cc_out"], roff, dram_pat))


    def rev_t(ap2d, nrows):
        # view [rows, t, y] with t reversed
        return AP(ap2d.tensor, ap2d.offset + (T - 1) * NTAGS,
                  [list(ap2d.ap[0]), [-NTAGS, T], [1, NTAGS]])

    v3 = lambda a: a.rearrange("p (t y) -> p t y", y=NTAGS)
    nc.vector.tensor_add(v3(emit_sb[0:8, :]), v3(s1g[0:8, :]), rev_t(s1r[0:8, :], 8))
    nc.vector.tensor_add(v3(emit_sb[BOFF:BOFF + 8, :]), rev_t(s1g[BOFF:BOFF + 8, :], 8), v3(s1r[BOFF:BOFF + 8, :]))

    # ---------------- phase 7: the two max-plus scans ----------------

    CH = 32  # A-table chunk length (t steps)
    with tc.tile_pool(name="scanA", bufs=2) as apool, \
         tc.tile_pool(name="scan", bufs=3) as scan_pool:
        for c0_ in range(0, T, CH):
            # A[j, t, next, prev] = trans_dual[j, next, prev] + emit[j, t, next]
            A = apool.tile([J, CH * 144], F32, tag="A")
            nc.gpsimd.tensor_tensor(
                out=_ap(A[:], 0, [[144, CH], [NTAGS, NTAGS], [1, NTAGS]]),
                in0=_ap(trans_sb[:], 0, [[0, CH], [NTAGS, NTAGS], [1, NTAGS]]),
                in1=_ap(emit_sb[:], c0_ * NTAGS, [[NTAGS, CH], [1, NTAGS], [0, NTAGS]]),
                op=ALU.add)
            for tt in range(CH):
                t = c0_ + tt
                m_t = scan_pool.tile([J, 144], F32, tag="m")
                fv_b = _ap(fv_all[:], t * NTAGS, [[0, NTAGS], [1, NTAGS]])
                nc.vector.tensor_add(
                    m_t[:].rearrange("p (a b) -> p a b", b=NTAGS), fv_b,
                    _ap(A[:], tt * 144, [[NTAGS, NTAGS], [1, NTAGS]]))
                nc.vector.reduce_max(fv_all[:, (t + 1) * NTAGS:(t + 2) * NTAGS],
                                     m_t[:].rearrange("p (a b) -> p a b", b=NTAGS),
                                     axis=mybir.AxisListType.X)

    if upto < 8:
        nc.gpsimd.dma_start(AP(io["paths"], 0, [[1, 64]]), fv_all[0:1, 0:64])
        return
    # ---------------- phase 8: finalize ----------------
    u2 = vit.tile([BC, T * NTAGS], F32, tag="u2")
    nc.sync.dma_start(u2[:], fv_all[BOFF:BOFF + 8, NTAGS:(T + 1) * NTAGS])

    tA = vit.tile([BC, T * NTAGS], F32, tag="tA")
    nc.vector.tensor_sub(tA[:], fv_all[0:8, NTAGS:(T + 1) * NTAGS], emit_sb[0:8, :])
    tot = vit.tile([BC, T * NTAGS], F32, tag="tot")
    nc.vector.tensor_add(v3(tot[:]), v3(tA[:]), rev_t(u2[:], 8))

    maxv = vit.tile([BC, T], F32, tag="maxv")
    nc.vector.reduce_max(maxv[:], v3(tot[:]), axis=mybir.AxisListType.X)

    eq = vit.tile([BC, T * NTAGS], F32, tag="s1g")
    nc.vector.tensor_tensor(
        out=v3(eq[:]), in0=v3(tot[:]),
        in1=_ap(maxv[:], 0, [[1, T], [0, NTAGS]]),
        op=ALU.is_ge)
    masked = vit.tile([BC, T * NTAGS], F32, tag="s1r")
    nc.vector.scalar_tensor_tensor(
        out=masked[:], in0=eq[:], scalar=1.0e6, in1=iota_f[:],
        op0=ALU.mult, op1=ALU.subtract)
    pm = vit.tile([BC, T], F32, tag="pm")
    nc.vector.reduce_max(pm[:], v3(masked[:]), axis=mybir.AxisListType.X)
    pathf = vit.tile([BC, T], F32, tag="pathf")
    nc.vector.tensor_scalar(out=pathf[:], in0=pm[:], scalar1=-1.0, scalar2=1.0e6,
                            op0=ALU.mult, op1=ALU.add)
    paths_sb = vit.tile([BC, T], I32, tag="paths")
    nc.vector.tensor_copy(paths_sb[:], pathf[:])
    scores_sb = vit.tile([BC, 1], F32, tag="scores")
    nc.vector.tensor_copy(scores_sb[:], maxv[:, 0:1])

    nc.sync.dma_start(io["paths"].ap(), paths_sb[:])
    nc.sync.dma_start(AP(io["scores"], 0, [[1, BC], [1, 1]]), scores_sb[:])


# ---------------------------------------------------------------------------
# host side
# ---------------------------------------------------------------------------

def _wT_blocks(w):
    """[4HH, K] weight -> [128, (ktiles*mtiles)*128] lhsT block layout."""
    kk = w.shape[1] // 128
    mm = w.shape[0] // 128
    wt = w.T.reshape(kk, 128, mm, 128)          # [k, r, m, c]
    return np.ascontiguousarray(wt.transpose(1, 0, 2, 3).reshape(128, kk * mm * 128))


def _make_inputs(inputs):
    sent = np.asarray(inputs["sentence"]).astype(np.int32)
    embed = np.asarray(inputs["embed"]).astype(np.float32)
    h0 = np.asarray(inputs["h0"]).astype(np.float32)
    c0 = np.asarray(inputs["c0"]).astype(np.float32)
    trans = np.asarray(inputs["transitions"]).astype(np.float32)
    b_out = np.asarray(inputs["b_out"]).astype(np.float32)
    W_out = np.asarray(inputs["W_out"]).astype(np.float32)

    Wih = [np.asarray(inputs["W_ih_f"]), np.asarray(inputs["W_ih_b"])]
    Whh = [np.asarray(inputs["W_hh_f"]), np.asarray(inputs["W_hh_b"])]
    bb = [np.asarray(inputs["b_f"]), np.asarray(inputs["b_b"])]

    init_fv = np.full((J, NTAGS), NEG, np.float32)
    init_fv[0:8, START_IDX] = 0.0
    init_fv[32:40, STOP_IDX] = 0.0
    trans_dual = np.tile(trans.reshape(1, 144), (J, 1)).astype(np.float32)
    trans_dual[32:40] = np.tile(trans.T.reshape(1, 144), (8, 1))

    in_maps = []
    for c in range(NCORES):
        d = 1 if c >= 4 else 0
        p = c % 4
        sl = slice(8 * p, 8 * p + 8)
        s = sent[sl]
        if d == 1:
            s = s[:, ::-1]
        flat = np.ascontiguousarray(s.T).reshape(TOK)          # t-major, b-minor
        idx = np.ascontiguousarray(flat.reshape(16, 128).T).astype(np.int32)

        # tanh-trick scaling: i,f,o rows x1/2 (sigmoid via tanh); and the
        # device carries h' = 2h, v = 2c -> W_hh cols x1/2, W_out x1/2, h0 x2, c0 x2.
        rs = np.full((1024, 1), 0.5, np.float32)
        rs[512:768] = 1.0  # g rows keep full scale (plain tanh)
        wih = Wih[d][GATE_PERM] * rs
        whh = Whh[d][GATE_PERM] * rs * 0.5
        bvec = bb[d][GATE_PERM] * rs[:, 0]
        h0s = h0[d][sl] * 2.0   # [8, HH]
        c0s = c0[d][sl] * 2.0
        h0T = np.ascontiguousarray(h0s.T.reshape(KH, 128, 8).transpose(1, 0, 2)
                                   .reshape(128, 16))
        c0T = np.ascontiguousarray(c0s.T.reshape(KH, 128, 8).transpose(1, 0, 2)
                                   .reshape(128, 16))
        wout = W_out[:, d * HH:(d + 1) * HH] * 0.5  # [12, 256]; x1/2 since h'=2h
        woutT = np.ascontiguousarray(wout.T.reshape(KH, 128, NTAGS)
                                     .transpose(1, 0, 2).reshape(128, KH * NTAGS))
        bout_rep = (np.tile(b_out, (128, 1)) if d == 0
                    else np.zeros((128, NTAGS))).astype(np.float32)

        in_maps.append({
            "embed": embed,
            "idx": idx,
            "wihT": _wT_blocks(wih).astype(ml_dtypes.bfloat16),
            "whhT": _wT_blocks(whh).astype(ml_dtypes.bfloat16),
            "woutT": woutT.astype(ml_dtypes.bfloat16),
            "bT": np.ascontiguousarray(bvec.reshape(M, 128).T).astype(np.float32),
            "bout_rep": bout_rep,
            "h0T": h0T.astype(ml_dtypes.bfloat16),
            "c0T": c0T.astype(np.float32),
            "trans_dual": trans_dual,
            "init_fv": init_fv,
        })
    return in_maps


_NC_CACHE = None


def _get_nc():
    global _NC_CACHE
    if _NC_CACHE is None:
        _NC_CACHE = build_program()
    return _NC_CACHE


def kernel(**inputs):
    nc = _get_nc()
    in_maps = _make_inputs(inputs)
    res = run_bass_kernel_spmd(nc, in_maps, core_ids=list(range(NCORES)))
    scores = np.zeros(B, np.float32)
    paths = np.zeros((B, T), np.int32)
    for p in range(4):
        r = res.results[p]
        scores[8 * p:8 * p + 8] = r["out_scores"]
        paths[8 * p:8 * p + 8] = r["out_paths"]
    return scores, paths


if __name__ == "__main__":
    nc = _get_nc()
    print("program built + compiled OK")


# revision 38
# speedup vs baseline: 1.0525x; 1.0057x over previous
"""BiLSTM-CRF (Viterbi decode) Trainium2 Bass kernel.

Sharding: direction-split x batch-split over 8 cores.
  - Pair p in {0,1,2,3}: cores p (forward LSTM) and p+4 (backward LSTM, fed
    time-reversed tokens) both own sentences [8p, 8p+8).
  - Each core: gathers embeddings on-device (indirect DMA), precomputes the
    input contribution Zx = W_ih @ x_t for all t (bf16 matmul), runs the
    sequential LSTM recurrence with stationary bf16 weights (gates land
    transposed: [gate_dim on partitions, batch on free] -> no per-step
    transpose), projects to CRF emission scores, and AllGathers the partial
    emissions within its pair.
  - Viterbi runs WITHOUT traceback: a forward max-plus scan and a backward
    max-plus scan (16 jobs in one set of [16, 144] DVE ops), then
    path[t] = argmax_y(fwd[t,y] + bwd[T-1-t,y] - emit[t,y]) fully in parallel.
    Both cores of a pair redundantly compute all 8 sentences; host reads
    outputs from cores 0-3.
"""

import os
import sys

for _p in ("/opt/trn_rl_repo", "/root/.axon_site/_ro/trn_rl_repo"):
    if os.path.isdir(_p) and _p not in sys.path:
        sys.path.append(_p)

import numpy as np
import ml_dtypes

import concourse.bass as bass
import concourse.tile as tile
from concourse import bacc, mybir
from concourse.bass import AP, IndirectOffsetOnAxis
from concourse.bass_utils import run_bass_kernel_spmd
from concourse.masks import make_identity

F32 = mybir.dt.float32
BF16 = mybir.dt.bfloat16
I32 = mybir.dt.int32
AF = mybir.ActivationFunctionType
ALU = mybir.AluOpType

VOCAB = 100000
EMB = 256
HID = 512
HH = 256  # per-direction hidden
NTAGS = 12
START_IDX = 10
STOP_IDX = 11
NEG = -10000.0

B = 32
T = 256
NCORES = 8
BC = 8          # sentences per pair/core
TOK = T * BC    # 2048 tokens per core
KE = 2          # emb k-tiles
KH = 2          # hidden k-tiles
M = 8           # gate m-tiles (4*HH/128)
J = 40          # viterbi scan partition span: fwd jobs rows 0-7, bwd jobs rows 32-39
BOFF = 32       # partition offset of backward jobs (engine APs need 32-aligned starts)
HSLOT = T + 1   # h history slots (slot 0 = h0)
HFREE = KH * HSLOT * 8  # h_allT free size

# gate order stays (i, f, g, o); with the tanh-trick no contiguity is needed
GATE_PERM = np.arange(1024)

REPLICA_GROUPS = [[0, 4], [1, 5], [2, 6], [3, 7]]


def _ap(t_ap, offset, pattern):
    """New AP over the same tensor with explicit free pattern (keeps partition dim)."""
    return AP(t_ap.tensor, offset, [list(t_ap.ap[0])] + [list(p) for p in pattern])


def build_program(fake_cc=False, num_devices=NCORES, upto=99):
    nc = bacc.Bacc("TRN2", target_bir_lowering=False, debug=False,
                   enable_asserts=False, num_devices=num_devices)

    # ---- I/O ----
    embed_d = nc.dram_tensor("embed", [VOCAB, EMB], F32, kind="ExternalInput")
    idx_d = nc.dram_tensor("idx", [128, 16], I32, kind="ExternalInput")
    wihT_d = nc.dram_tensor("wihT", [128, 2048], BF16, kind="ExternalInput")
    whhT_d = nc.dram_tensor("whhT", [128, 2048], BF16, kind="ExternalInput")
    woutT_d = nc.dram_tensor("woutT", [128, 24], BF16, kind="ExternalInput")
    bT_d = nc.dram_tensor("bT", [128, M], F32, kind="ExternalInput")
    bout_d = nc.dram_tensor("bout_rep", [128, NTAGS], F32, kind="ExternalInput")
    h0T_d = nc.dram_tensor("h0T", [128, 16], BF16, kind="ExternalInput")
    c0T_d = nc.dram_tensor("c0T", [128, 16], F32, kind="ExternalInput")
    trans_d = nc.dram_tensor("trans_dual", [J, 144], F32, kind="ExternalInput")
    init_d = nc.dram_tensor("init_fv", [J, NTAGS], F32, kind="ExternalInput")

    scores_d = nc.dram_tensor("out_scores", [BC], F32, kind="ExternalOutput")
    paths_d = nc.dram_tensor("out_paths", [BC, T], I32, kind="ExternalOutput")

    # collective bounce buffers
    cc_in = nc.dram_tensor("cc_in", [TOK * NTAGS], F32, kind="Internal")
    cc_out = nc.dram_tensor("cc_out", [2 * TOK * NTAGS], F32, kind="Internal")

    from contextlib import ExitStack
    with tile.TileContext(nc) as tc, ExitStack() as ctx:
        _emit(ctx, tc, dict(embed=embed_d, idx=idx_d, wihT=wihT_d, whhT=whhT_d,
                            woutT=woutT_d, bT=bT_d, bout=bout_d, h0T=h0T_d,
                            c0T=c0T_d, trans=trans_d, init=init_d,
                            scores=scores_d, paths=paths_d,
                            cc_in=cc_in, cc_out=cc_out), fake_cc=fake_cc, upto=upto)
    nc.compile()
    return nc


def _emit(ctx, tc, io, fake_cc=False, upto=99):
    nc = tc.nc

    const = ctx.enter_context(tc.tile_pool(name="const", bufs=1))

    ident = const.tile([128, 128], F32, tag="ident")
    make_identity(nc, ident[:])

    # iota over y, [8, T, NTAGS] (values 0..11 are exact in f32)
    iota_f = const.tile([BC, T * NTAGS], F32, tag="iota_f")
    nc.gpsimd.iota(iota_f[:].rearrange("p (t y) -> p t y", y=NTAGS),
                   pattern=[[0, T], [1, NTAGS]], base=0, channel_multiplier=0,
                   allow_small_or_imprecise_dtypes=True)

    # ---- load params ----
    wihT = const.tile([128, 2048], BF16, tag="wihT")
    nc.sync.dma_start(wihT[:], io["wihT"].ap())
    whhT = const.tile([128, 2048], BF16, tag="whhT")
    nc.sync.dma_start(whhT[:], io["whhT"].ap())
    woutT = const.tile([128, 24], BF16, tag="woutT")
    nc.sync.dma_start(woutT[:], io["woutT"].ap())
    bT = const.tile([128, M], F32, tag="bT")
    nc.sync.dma_start(bT[:], io["bT"].ap())
    bout = const.tile([128, NTAGS], F32, tag="bout")
    nc.sync.dma_start(bout[:], io["bout"].ap())
    idx_sb = const.tile([128, 16], I32, tag="idx")
    nc.sync.dma_start(idx_sb[:], io["idx"].ap())
    trans_sb = const.tile([J, 144], F32, tag="trans")
    nc.sync.dma_start(trans_sb[:], io["trans"].ap())
    c0T = const.tile([128, 16], F32, tag="c0T")
    nc.sync.dma_start(c0T[:], io["c0T"].ap())

    # h history: [128, KH * HSLOT * 8] bf16; slot 0 = h0
    hall_pool = ctx.enter_context(tc.tile_pool(name="hall", bufs=1))
    h_allT = hall_pool.tile([128, HFREE], BF16, tag="h_allT")
    nc.sync.dma_start(
        _ap(h_allT[:], 0, [[HSLOT * 8, KH], [1, 8]]),
        AP(io["h0T"], 0, [[16, 128], [8, KH], [1, 8]]))

    # emit/fv live in a pre-recurrence pool whose addresses never overlap the
    # zx/x pools, so their big memsets + init DMA run during the gather phase
    # instead of serializing at the recurrence->Viterbi junction.
    early = ctx.enter_context(tc.tile_pool(name="vearly", bufs=1))
    emit_sb = early.tile([J, T * NTAGS], F32, tag="emit")
    nc.vector.memset(emit_sb[:], 0.0)
    fv_all = early.tile([J, (T + 1) * NTAGS], F32, tag="fv_all")
    nc.vector.memset(fv_all[:], NEG)
    nc.sync.dma_start(fv_all[:, 0:NTAGS], io["init"].ap())

    # ---------------- phases 1-4 (pools released before Viterbi) ----------------
    # All of gather/transpose/Zx is split into 4 token-chunks (512 tokens = 64
    # steps each) held in separate tiles, so the recurrence can start as soon
    # as chunk 0 is ready instead of waiting for the whole serial prefix.
    NCHUNK, CTOK = 4, 512
    zx_ctx = tc.tile_pool(name="zx", bufs=1)
    zx_pool = zx_ctx.__enter__()
    zxs = [zx_pool.tile([128, M * CTOK], F32, tag=f"zx{n}", name=f"zxc{n}")
           for n in range(NCHUNK)]
    with tc.tile_pool(name="xpool", bufs=1) as xpool, \
         tc.tile_pool(name="xt", bufs=1) as xtpool, \
         tc.tile_pool(name="tp_ps", bufs=4, space="PSUM") as tp_ps, \
         tc.tile_pool(name="zx_ps", bufs=4, space="PSUM") as zx_ps:
        for n in range(NCHUNK):
            xr = xpool.tile([128, 4 * EMB], F32, tag=f"xr{n}")
            for jj in range(4):
                j = n * 4 + jj
                nc.gpsimd.indirect_dma_start(
                    out=xr[:, jj * EMB:(jj + 1) * EMB],
                    out_offset=None,
                    in_=io["embed"].ap(),
                    in_offset=IndirectOffsetOnAxis(ap=idx_sb[:, j:j + 1], axis=0))
            xt = xtpool.tile([128, KE * CTOK], BF16, tag=f"xt{n}")
            for jj in range(4):
                for h in range(KE):
                    ps = tp_ps.tile([128, 128], F32, tag="tp")
                    nc.tensor.transpose(
                        out=ps[:], in_=xr[:, jj * EMB + h * 128: jj * EMB + (h + 1) * 128],
                        identity=ident[:])
                    nc.vector.tensor_copy(
                        xt[:, h * CTOK + jj * 128: h * CTOK + (jj + 1) * 128], ps[:])
            if upto < 3:
                continue
            for m in range(M):
                ps = zx_ps.tile([128, 512], F32, tag="zx")
                for k in range(KE):
                    nc.tensor.matmul(
                        out=ps[:],
                        lhsT=wihT[:, (k * M + m) * 128:(k * M + m + 1) * 128],
                        rhs=xt[:, k * CTOK:(k + 1) * CTOK],
                        start=(k == 0), stop=(k == KE - 1))
                dst = zxs[n][:, m * CTOK:(m + 1) * CTOK]
                if (m + n) % 2 == 0:
                    nc.scalar.activation(out=dst, in_=ps[:], func=AF.Identity,
                                         bias=bT[:, m:m + 1], scale=1.0)
                else:
                    nc.vector.tensor_scalar_add(dst, ps[:], bT[:, m:m + 1])
        if upto < 3:
            nc.gpsimd.dma_start(AP(io["paths"], 0, [[1, 64]]), xt[0:1, 0:64])
            return

    if upto < 4:
        nc.gpsimd.dma_start(AP(io["paths"], 0, [[1, 64]]), zxs[0][0:1, 0:64])
        return
    # ---------------- phase 4: LSTM recurrence ----------------
    rec_ctxs = [tc.tile_pool(name="rec_ps", bufs=4, space="PSUM"),
                tc.tile_pool(name="gates", bufs=4),
                tc.tile_pool(name="small", bufs=8),
                tc.tile_pool(name="cpool", bufs=4)]
    rec_ps, gates, small, cpool = [c.__enter__() for c in rec_ctxs]

    c_prev = cpool.tile([128, 16], F32, tag="c")
    nc.vector.tensor_copy(c_prev[:], c0T[:])

    zx_views = [z[:].rearrange("p (m t b) -> p m t b", m=M, b=BC)
                for z in zxs]
    for t in range(T):
        ps = rec_ps.tile([128, M * BC], F32, tag="rec")
        for m in range(M):
            for k in range(KH):
                nc.tensor.matmul(
                    out=ps[:, m * BC:(m + 1) * BC],
                    lhsT=whhT[:, (k * M + m) * 128:(k * M + m + 1) * 128],
                    rhs=h_allT[:, k * HSLOT * 8 + t * 8: k * HSLOT * 8 + (t + 1) * 8],
                    start=(k == 0), stop=(k == KH - 1))
        # gates via ONE tanh per half: i,f,o rows pre-scaled by 1/2 host-side, so
        # tanh(z/2) -> sigmoid(z) = (th+1)/2; g is a plain tanh. The cell state is
        # kept as v = 2c and hidden as h' = 2h (weights pre-scaled), which turns
        # every product into a fused (x+1)*y / (0.5x+y) scalar_tensor_tensor op.
        # The o-gate half (m-tiles 6,7) is processed separately so its add+tanh
        # overlaps the i/f/g -> v -> tanh(c) chain.
        gp = gates.tile([128, 48], F32, tag="gp")
        nc.vector.tensor_add(
            gp[:, 0:48].rearrange("p (m b) -> p m b", b=BC),
            ps[:, 0:48].rearrange("p (m b) -> p m b", b=BC),
            zx_views[t // 64][:, 0:6, t % 64, :])
        th = gates.tile([128, 48], F32, tag="th")
        nc.scalar.activation(out=th[:, 0:48], in_=gp[:, 0:48], func=AF.Tanh)
        u1 = small.tile([128, 16], F32, tag="u1")
        nc.vector.scalar_tensor_tensor(out=u1[:], in0=th[:, 16:32], scalar=1.0,
                                       in1=c_prev[:], op0=ALU.add, op1=ALU.mult)
        u2 = small.tile([128, 16], F32, tag="u2")
        nc.vector.scalar_tensor_tensor(out=u2[:], in0=th[:, 0:16], scalar=1.0,
                                       in1=th[:, 32:48], op0=ALU.add, op1=ALU.mult)
        c_new = cpool.tile([128, 16], F32, tag="c")
        nc.vector.scalar_tensor_tensor(out=c_new[:], in0=u1[:], scalar=0.5,
                                       in1=u2[:], op0=ALU.mult, op1=ALU.add)
        tc_t = small.tile([128, 16], F32, tag="tc")
        nc.scalar.activation(out=tc_t[:], in_=c_new[:], func=AF.Tanh, scale=0.5)
        gp2 = small.tile([128, 16], F32, tag="gp2")
        nc.vector.tensor_add(
            gp2[:].rearrange("p (m b) -> p m b", b=BC),
            ps[:, 48:64].rearrange("p (m b) -> p m b", b=BC),
            zx_views[t // 64][:, 6:8, t % 64, :])
        th2 = small.tile([128, 16], F32, tag="th2")
        nc.scalar.activation(out=th2[:], in_=gp2[:], func=AF.Tanh)
        hout = _ap(h_allT[:], (t + 1) * 8, [[HSLOT * 8, KH], [1, 8]])
        nc.vector.scalar_tensor_tensor(
            out=hout,
            in0=th2[:].rearrange("p (k b) -> p k b", b=8), scalar=1.0,
            in1=tc_t[:].rearrange("p (k b) -> p k b", b=8),
            op0=ALU.add, op1=ALU.mult)
        c_prev = c_new

    for c in reversed(rec_ctxs):
        c.__exit__(None, None, None)
    zx_ctx.__exit__(None, None, None)

    if upto < 5:
        nc.gpsimd.dma_start(AP(io["paths"], 0, [[1, 64]]), h_allT[0:1, 0:128].bitcast(I32))
        return
    # ---------------- phase 5: emission partials + AllGather ----------------
    vit = ctx.enter_context(tc.tile_pool(name="vit", bufs=1))
    feats_sb = vit.tile([128, 16 * NTAGS], F32, tag="feats")
    with tc.tile_pool(name="f_ps", bufs=4, space="PSUM") as f_ps:
        for mc in range(16):
            ps = f_ps.tile([128, NTAGS], F32, tag="f")
            for k in range(KH):
                nc.tensor.matmul(
                    out=ps[:],
                    lhsT=h_allT[:, k * HSLOT * 8 + 8 + mc * 128: k * HSLOT * 8 + 8 + (mc + 1) * 128],
                    rhs=woutT[:, k * NTAGS:(k + 1) * NTAGS],
                    start=(k == 0), stop=(k == KH - 1))
            nc.vector.tensor_add(feats_sb[:, mc * NTAGS:(mc + 1) * NTAGS],
                                 ps[:], bout[:])

    # SBUF [128 p, (chunk 16, y 12)] -> DRAM flat [(chunk*128+p) * 12 + y]
    nc.sync.dma_start(
        AP(io["cc_in"], 0, [[NTAGS, 128], [128 * NTAGS, 16], [1, NTAGS]]),
        feats_sb[:].rearrange("p (c y) -> p c y", y=NTAGS))
    if fake_cc:
        # single-core timing-model variant: stand in for the pair AllGather
        half = TOK * NTAGS
        nc.sync.dma_start(AP(io["cc_out"], 0, [[1, half]]), io["cc_in"].ap())
        nc.sync.dma_start(AP(io["cc_out"], half, [[1, half]]), io["cc_in"].ap())
    else:
        nc.gpsimd.collective_compute(
            "AllGather", ALU.bypass, replica_groups=REPLICA_GROUPS,
            ins=[io["cc_in"].ap()], outs=[io["cc_out"].ap()])

    # ---------------- phase 6: build emit_sb [J, T*12] ----------------
    # G = rank0 partial (global t order), R = rank1 partial (reversed t order)
    s1g = vit.tile([J, T * NTAGS], F32, tag="s1g")
    s1r = vit.tile([J, T * NTAGS], F32, tag="s1r")
    goff, roff = 0, TOK * NTAGS
    dram_pat = [[NTAGS, BC], [BC * NTAGS, T], [1, NTAGS]]  # (b, t, y)
    nc.sync.dma_start(s1g[0:8, :], AP(io["cc_out"], goff, dram_pat))
    nc.sync.dma_start(s1g[BOFF:BOFF + 8, :], AP(io["cc_out"], goff, dram_pat))
    nc.sync.dma_start(s1r[0:8, :], AP(io["cc_out"], roff, dram_pat))
    nc.sync.dma_start(s1r[BOFF:BOFF + 8, :], AP(io["cc_out"], roff, dram_pat))


    def rev_t(ap2d, nrows):
        # view [rows, t, y] with t reversed
        return AP(ap2d.tensor, ap2d.offset + (T - 1) * NTAGS,
                  [list(ap2d.ap[0]), [-NTAGS, T], [1, NTAGS]])

    v3 = lambda a: a.rearrange("p (t y) -> p t y", y=NTAGS)
    nc.vector.tensor_add(v3(emit_sb[0:8, :]), v3(s1g[0:8, :]), rev_t(s1r[0:8, :], 8))
    nc.vector.tensor_add(v3(emit_sb[BOFF:BOFF + 8, :]), rev_t(s1g[BOFF:BOFF + 8, :], 8), v3(s1r[BOFF:BOFF + 8, :]))

    # ---------------- phase 7: the two max-plus scans ----------------

    CH = 8  # A-table chunk length (t steps)
    with tc.tile_pool(name="scanA", bufs=2) as apool, \
         tc.tile_pool(name="scan", bufs=3) as scan_pool:
        for c0_ in range(0, T, CH):
            # A[j, t, next, prev] = trans_dual[j, next, prev] + emit[j, t, next]
            A = apool.tile([J, CH * 144], F32, tag="A")
            nc.gpsimd.tensor_tensor(
                out=_ap(A[:], 0, [[144, CH], [NTAGS, NTAGS], [1, NTAGS]]),
                in0=_ap(trans_sb[:], 0, [[0, CH], [NTAGS, NTAGS], [1, NTAGS]]),
                in1=_ap(emit_sb[:], c0_ * NTAGS, [[NTAGS, CH], [1, NTAGS], [0, NTAGS]]),
                op=ALU.add)
            for tt in range(CH):
                t = c0_ + tt
                m_t = scan_pool.tile([J, 144], F32, tag="m")
                fv_b = _ap(fv_all[:], t * NTAGS, [[0, NTAGS], [1, NTAGS]])
                nc.vector.tensor_add(
                    m_t[:].rearrange("p (a b) -> p a b", b=NTAGS), fv_b,
                    _ap(A[:], tt * 144, [[NTAGS, NTAGS], [1, NTAGS]]))
                nc.vector.reduce_max(fv_all[:, (t + 1) * NTAGS:(t + 2) * NTAGS],
                                     m_t[:].rearrange("p (a b) -> p a b", b=NTAGS),
                                     axis=mybir.AxisListType.X)

    if upto < 8:
        nc.gpsimd.dma_start(AP(io["paths"], 0, [[1, 64]]), fv_all[0:1, 0:64])
        return
    # ---------------- phase 8: finalize ----------------
    u2 = vit.tile([BC, T * NTAGS], F32, tag="u2")
    nc.sync.dma_start(u2[:], fv_all[BOFF:BOFF + 8, NTAGS:(T + 1) * NTAGS])

    tA = vit.tile([BC, T * NTAGS], F32, tag="tA")
    nc.vector.tensor_sub(tA[:], fv_all[0:8, NTAGS:(T + 1) * NTAGS], emit_sb[0:8, :])
    tot = vit.tile([BC, T * NTAGS], F32, tag="tot")
    nc.vector.tensor_add(v3(tot[:]), v3(tA[:]), rev_t(u2[:], 8))

    maxv = vit.tile([BC, T], F32, tag="maxv")
    nc.vector.reduce_max(maxv[:], v3(tot[:]), axis=mybir.AxisListType.X)

    eq = vit.tile([BC, T * NTAGS], F32, tag="s1g")
    nc.vector.tensor_tensor(
        out=v3(eq[:]), in0=v3(tot[:]),
        in1=_ap(maxv[:], 0, [[1, T], [0, NTAGS]]),
        op=ALU.is_ge)
    masked = vit.tile([BC, T * NTAGS], F32, tag="s1r")
    nc.vector.scalar_tensor_tensor(
        out=masked[:], in0=eq[:], scalar=1.0e6, in1=iota_f[:],
        op0=ALU.mult, op1=ALU.subtract)
    pm = vit.tile([BC, T], F32, tag="pm")
    nc.vector.reduce_max(pm[:], v3(masked[:]), axis=mybir.AxisListType.X)
    pathf = vit.tile([BC, T], F32, tag="pathf")
    nc.vector.tensor_scalar(out=pathf[:], in0=pm[:], scalar1=-1.0, scalar2=1.0e6,
                            op0=ALU.mult, op1=ALU.add)
    paths_sb = vit.tile([BC, T], I32, tag="paths")
    nc.vector.tensor_copy(paths_sb[:], pathf[:])
    scores_sb = vit.tile([BC, 1], F32, tag="scores")
    nc.vector.tensor_copy(scores_sb[:], maxv[:, 0:1])

    nc.sync.dma_start(io["paths"].ap(), paths_sb[:])
    nc.sync.dma_start(AP(io["scores"], 0, [[1, BC], [1, 1]]), scores_sb[:])


# ---------------------------------------------------------------------------
# host side
# ---------------------------------------------------------------------------

def _wT_blocks(w):
    """[4HH, K] weight -> [128, (ktiles*mtiles)*128] lhsT block layout."""
    kk = w.shape[1] // 128
    mm = w.shape[0] // 128
    wt = w.T.reshape(kk, 128, mm, 128)          # [k, r, m, c]
    return np.ascontiguousarray(wt.transpose(1, 0, 2, 3).reshape(128, kk * mm * 128))


def _make_inputs(inputs):
    sent = np.asarray(inputs["sentence"]).astype(np.int32)
    embed = np.asarray(inputs["embed"]).astype(np.float32)
    h0 = np.asarray(inputs["h0"]).astype(np.float32)
    c0 = np.asarray(inputs["c0"]).astype(np.float32)
    trans = np.asarray(inputs["transitions"]).astype(np.float32)
    b_out = np.asarray(inputs["b_out"]).astype(np.float32)
    W_out = np.asarray(inputs["W_out"]).astype(np.float32)

    Wih = [np.asarray(inputs["W_ih_f"]), np.asarray(inputs["W_ih_b"])]
    Whh = [np.asarray(inputs["W_hh_f"]), np.asarray(inputs["W_hh_b"])]
    bb = [np.asarray(inputs["b_f"]), np.asarray(inputs["b_b"])]

    init_fv = np.full((J, NTAGS), NEG, np.float32)
    init_fv[0:8, START_IDX] = 0.0
    init_fv[32:40, STOP_IDX] = 0.0
    trans_dual = np.tile(trans.reshape(1, 144), (J, 1)).astype(np.float32)
    trans_dual[32:40] = np.tile(trans.T.reshape(1, 144), (8, 1))

    in_maps = []
    for c in range(NCORES):
        d = 1 if c >= 4 else 0
        p = c % 4
        sl = slice(8 * p, 8 * p + 8)
        s = sent[sl]
        if d == 1:
            s = s[:, ::-1]
        flat = np.ascontiguousarray(s.T).reshape(TOK)          # t-major, b-minor
        idx = np.ascontiguousarray(flat.reshape(16, 128).T).astype(np.int32)

        # tanh-trick scaling: i,f,o rows x1/2 (sigmoid via tanh); and the
        # device carries h' = 2h, v = 2c -> W_hh cols x1/2, W_out x1/2, h0 x2, c0 x2.
        rs = np.full((1024, 1), 0.5, np.float32)
        rs[512:768] = 1.0  # g rows keep full scale (plain tanh)
        wih = Wih[d][GATE_PERM] * rs
        whh = Whh[d][GATE_PERM] * rs * 0.5
        bvec = bb[d][GATE_PERM] * rs[:, 0]
        h0s = h0[d][sl] * 2.0   # [8, HH]
        c0s = c0[d][sl] * 2.0
        h0T = np.ascontiguousarray(h0s.T.reshape(KH, 128, 8).transpose(1, 0, 2)
                                   .reshape(128, 16))
        c0T = np.ascontiguousarray(c0s.T.reshape(KH, 128, 8).transpose(1, 0, 2)
                                   .reshape(128, 16))
        wout = W_out[:, d * HH:(d + 1) * HH] * 0.5  # [12, 256]; x1/2 since h'=2h
        woutT = np.ascontiguousarray(wout.T.reshape(KH, 128, NTAGS)
                                     .transpose(1, 0, 2).reshape(128, KH * NTAGS))
        bout_rep = (np.tile(b_out, (128, 1)) if d == 0
                    else np.zeros((128, NTAGS))).astype(np.float32)

        in_maps.append({
            "embed": embed,
            "idx": idx,
            "wihT": _wT_blocks(wih).astype(ml_dtypes.bfloat16),
            "whhT": _wT_blocks(whh).astype(ml_dtypes.bfloat16),
            "woutT": woutT.astype(ml_dtypes.bfloat16),
            "bT": np.ascontiguousarray(bvec.reshape(M, 128).T).astype(np.float32),
            "bout_rep": bout_rep,
            "h0T": h0T.astype(ml_dtypes.bfloat16),
            "c0T": c0T.astype(np.float32),
            "trans_dual": trans_dual,
            "init_fv": init_fv,
        })
    return in_maps


_NC_CACHE = None


def _get_nc():
    global _NC_CACHE
    if _NC_CACHE is None:
        _NC_CACHE = build_program()
    return _NC_CACHE


def kernel(**inputs):
    nc = _get_nc()
    in_maps = _make_inputs(inputs)
    res = run_bass_kernel_spmd(nc, in_maps, core_ids=list(range(NCORES)))
    scores = np.zeros(B, np.float32)
    paths = np.zeros((B, T), np.int32)
    for p in range(4):
        r = res.results[p]
        scores[8 * p:8 * p + 8] = r["out_scores"]
        paths[8 * p:8 * p + 8] = r["out_paths"]
    return scores, paths


if __name__ == "__main__":
    nc = _get_nc()
    print("program built + compiled OK")
